# revision 26
# baseline (speedup 1.0000x reference)
"""MiniFastSpeech Trainium2 kernel (v6: 4-chain latency-hiding bf16 LSTM).

v3 (2 chains/core) measured loop-bound: the per-step recurrence
dependency chain (mms -> sigmoid -> DVE c-update -> tanh(c) -> h-write ->
mms) is ~4.5us while engine busy is only ~3.2us/step -- ~1.8us/step of
semaphore/pipeline dead time that scheduling cannot remove (every
DVE-produced value costs ~420ns to reach its consumer).

v6 goes busy-bound instead: 4 chains per core (2 fwd + 2 bwd, each 128
lanes = 2 seq-chunks x 64 batch; 32 chunks per direction, CHUNK=21,
W=12 warmup). The period must cover 4 chains' engine work (~6.4us on
Act) which exceeds the ~4.5us chain loop, so the recurrence latency
hides completely. Act work per chain-step: sigmoid [512] over bank A =
[i,f], tanh(x/2) [512] over bank B = [g,o] (g rows pre-doubled -> exact
tanh(g)), tanh [256] of c. DVE (bf16 2x): fc, ig, c_new tensor ops +
2 scalar_tensor_tensor h-writes computing h2 = (tanh(o/2)+1)*tanh(c) =
2h (whh/lin pre-halved on host absorb the 2x). PSUM: 8 banks = 4 chains
x 2 banks, bufs=1; xe matmuls run in-step (no prefetch; PE has slack).
Phase-2 final linear per chain-pair interleaved into the loop; bias add
on Pool.
"""

import sys
import numpy as np
from contextlib import ExitStack

sys.path.insert(0, "/opt/trn_rl_repo")

import concourse.bass as bass
import concourse.tile as tile
from concourse import bacc, mybir
from concourse.bass_utils import run_bass_kernel_spmd

# ---- problem constants (hardcoded per contract) ----
VOCAB, EMB, HID, MEL = 256, 128, 256, 80
B, T = 64, 512
N_CORES = 8
NCHUNK = 32          # chunks per direction
W = 11               # warmup steps per chain
CHUNK = 21           # positions per chunk; L_PAD = 672 >= L
L_PAD = NCHUNK * CHUNK
K_STEPS = W + CHUNK
F32 = mybir.dt.float32
BF16 = mybir.dt.bfloat16
SIG = mybir.ActivationFunctionType.Sigmoid
TANH = mybir.ActivationFunctionType.Tanh
MULT = mybir.AluOpType.mult
ADD = mybir.AluOpType.add

_COMPILED = None


def _host_expand(x, embed, dp_w, dp_b):
    xe = embed[x]                                   # (B,T,E)
    d = np.maximum(xe @ dp_w[0] + dp_b[0], 0)
    dur = np.floor(d).astype(np.int64) + 1
    cum = np.cumsum(dur, axis=1)
    L = int(cum[:, -1].max())
    pos = np.arange(L)
    idx = np.empty((B, L), np.int64)
    for b in range(B):
        idx[b] = np.searchsorted(cum[b], pos, side="right")
    mask = (pos[None, :] < cum[:, -1:]).astype(np.float32)
    exp = np.take_along_axis(xe, np.clip(idx, 0, T - 1)[..., None], axis=1)
    return np.ascontiguousarray(exp * mask[..., None], dtype=np.float32), L


# m-chunk order [i0 i1 f0 f1 | g0 g1 o0 o1]; rows in PyTorch [i,f,g,o] layout.
# Bank A = [i,f] -> one sigmoid act [512]. Bank B = [g,o] -> one tanh(x*0.5)
# act: g rows pre-doubled -> exact tanh(g) in cols 0:256; o gives tanh(o/2).
# whh: additionally all rows halved because the moving h operand is 2h.
def _mchunk_rows():
    rows, sc_ih, sc_hh = [], [], []
    for base, sc in ((0, 1.0), (HID, 1.0), (2 * HID, 2.0), (3 * HID, 1.0)):
        for half in (0, 1):
            rows.append(np.arange(base + half * 128, base + half * 128 + 128))
            sc_ih.append(np.full(128, sc, np.float32))
            sc_hh.append(np.full(128, sc * 0.5, np.float32))
    return (np.concatenate(rows), np.concatenate(sc_ih), np.concatenate(sc_hh))


class _Chain:
    def __init__(self, name, whh, wih, xe_cols, X, poolA, poolB):
        self.name = name
        self.whh = whh          # sbuf [128, 16*128] bf16, tile (m,k) at (2m+k)*128
        self.wih = wih          # sbuf [128, 8*128] bf16, tile m at m*128
        self.xe_cols = xe_cols  # slice in the xein tile
        self.X = X              # sbuf [128, 2*XW] bf16; k-half at k*XW
        self.poolA = poolA
        self.poolB = poolB
        self.gA = None
        self.gB = None
        self.src = None         # (h0, h1) col blocks [128,128] (2h of prev step)
        self.c_prev = None
        self.sf = None
        self.tB = None
        self.fc = None
        self.ig = None
        self.c_new = None
        self.tc = None


def _build_kernel():
    nc = bacc.Bacc("TRN2", target_bir_lowering=False, debug=False,
                   num_devices=N_CORES)

    xein = nc.dram_tensor("xein", [K_STEPS, EMB, 512], BF16,
                          kind="ExternalInput").ap()
    whh_f_d = nc.dram_tensor("whhT_f", [128, 16 * 128], BF16, kind="ExternalInput").ap()
    whh_b_d = nc.dram_tensor("whhT_b", [128, 16 * 128], BF16, kind="ExternalInput").ap()
    wih_f_d = nc.dram_tensor("wihT_f", [128, 8 * 128], BF16, kind="ExternalInput").ap()
    wih_b_d = nc.dram_tensor("wihT_b", [128, 8 * 128], BF16, kind="ExternalInput").ap()
    lin_w_d = nc.dram_tensor("linT", [128, 4 * MEL], BF16, kind="ExternalInput").ap()
    lin_b_d = nc.dram_tensor("lin_b", [MEL, 1], F32, kind="ExternalInput").ap()
    out_d = nc.dram_tensor("out_mel", [MEL, 2, CHUNK, 2, B], F32,
                           kind="ExternalOutput").ap()

    with tile.TileContext(nc) as tc, ExitStack() as ctx:
        wpool = ctx.enter_context(tc.tile_pool(name="weights", bufs=1))
        xpool = ctx.enter_context(tc.tile_pool(name="xstream", bufs=4))
        state = ctx.enter_context(tc.tile_pool(name="state", bufs=3))
        actp = ctx.enter_context(tc.tile_pool(name="acts", bufs=3))
        xbig = ctx.enter_context(tc.tile_pool(name="xbig", bufs=1))
        scr = ctx.enter_context(tc.tile_pool(name="scratch", bufs=3))
        # one pool per (pair, bank-type); tiles span 2 PSUM banks so the
        # pair's two chains share one [128,1024] gate tile -> merged acts.
        psA = [ctx.enter_context(tc.tile_pool(name=f"gA{i}", bufs=1,
                                              space="PSUM")) for i in range(2)]
        psB = [ctx.enter_context(tc.tile_pool(name=f"gB{i}", bufs=1,
                                              space="PSUM")) for i in range(2)]
        ostage = ctx.enter_context(tc.tile_pool(name="ostage", bufs=2))

        # ---- memsets first (Pool queue) so the PE pre-warm starts at t~0
        hinit = wpool.tile([128, 256], BF16, tag="hinit")
        nc.gpsimd.memset(hinit[:], 0.0)
        zstat_bf = wpool.tile([128, 64], BF16, tag="zstatbf")
        nc.gpsimd.memset(zstat_bf[:], 0.0)

        # PE p-state pre-warm: burn the ramp on dummy matmuls while the
        # weight DMAs are in flight, so step 0 runs at full clock.
        warm = psB[1].tile([128, 1024], F32, tag="g", name="pewarm")
        NWARM = 10
        for i in range(NWARM):
            nc.tensor.matmul(warm[0:64, 0:256], zstat_bf[:], hinit[:],
                             start=(i == 0), stop=(i == NWARM - 1))

        # ---- xe stream DMAs ----
        xe_tiles = {}

        def emit_dma(s):
            if s not in xe_tiles and s < K_STEPS:
                xe = xpool.tile([EMB, 512], BF16, tag="xe", name=f"xe{s}")
                nc.sync.dma_start(xe[:], xein[s])
                xe_tiles[s] = xe

        emit_dma(0)
        emit_dma(1)

        # ---- weights -> SBUF
        wih_f = wpool.tile([128, 8 * 128], BF16, tag="wihf")
        nc.scalar.dma_start(wih_f[:], wih_f_d[:])
        wih_b = wpool.tile([128, 8 * 128], BF16, tag="wihb")
        nc.gpsimd.dma_start(wih_b[:], wih_b_d[:])
        whh_f = wpool.tile([128, 16 * 128], BF16, tag="whhf")
        nc.sync.dma_start(whh_f[:], whh_f_d[:])
        whh_b = wpool.tile([128, 16 * 128], BF16, tag="whhb")
        nc.scalar.dma_start(whh_b[:], whh_b_d[:])
        lin_w = wpool.tile([128, 4 * MEL], BF16, tag="linw")
        nc.scalar.dma_start(lin_w[:], lin_w_d[:])
        lin_b = wpool.tile([MEL, 1], F32, tag="linb")
        nc.gpsimd.dma_start(lin_b[:], lin_b_d[:])

        XW = (CHUNK + 2) * 128
        Xs = [xbig.tile([128, 2 * XW], BF16, tag=f"X{i}", name=f"X{i}")
              for i in range(4)]

        # chains: FA, BA, FB, BB (emit order); xe col blocks FA 0:128,
        # FB 128:256, BA 256:384, BB 384:512. Pair p = (chains[2p],
        # chains[2p+1]) shares 2-bank gate tiles from psA[p]/psB[p].
        chains = [
            _Chain("FA", whh_f, wih_f, slice(0, 128), Xs[0], psA[0], psB[0]),
            _Chain("BA", whh_b, wih_b, slice(256, 384), Xs[1], psA[0], psB[0]),
            _Chain("FB", whh_f, wih_f, slice(128, 256), Xs[2], psA[1], psB[1]),
            _Chain("BB", whh_b, wih_b, slice(384, 512), Xs[3], psA[1], psB[1]),
        ]
        for ch in chains:
            ch.src = (hinit[:, 0:128], hinit[:, 128:256])
            c0 = state.tile([128, 256], BF16, tag="c" + ch.name,
                            name=f"c0{ch.name}")
            nc.gpsimd.memset(c0[:], 0.0)
            ch.c_prev = c0

        gpair = {}

        def emit_mms(ch, s, pair, side, bank):
            # one bank's mms (xe + rec, k-major) for one chain. The pair's
            # two chains write halves of one [128,1024] 2-bank tile; each
            # half is a separate accumulation group (own start/stop) and a
            # separate PSUM bank, so starts don't clobber the neighbor.
            xe = xe_tiles[s]
            key = (pair, bank)
            if side == 0:
                pool = ch.poolA if bank == 0 else ch.poolB
                gpair[key] = pool.tile([128, 1024], F32, tag="g",
                                       name=f"g{bank}P{pair}_{s}")
            g = gpair[key][:, side * 512:(side + 1) * 512]
            mbase = 4 * bank
            for m in range(mbase, mbase + 4):
                col = (m % 4) * 128
                nc.tensor.matmul(g[:, col:col + 128],
                                 ch.wih[:, m * 128:(m + 1) * 128],
                                 xe[:, ch.xe_cols],
                                 start=(m == mbase),
                                 stop=(s == 0 and m == mbase + 3))
            if s > 0:
                for k in (0, 1):
                    for m in range(mbase, mbase + 4):
                        last = (m == mbase + 3) and (k == 1)
                        col = (m % 4) * 128
                        nc.tensor.matmul(
                            g[:, col:col + 128],
                            ch.whh[:, (2 * m + k) * 128:(2 * m + k + 1) * 128],
                            ch.src[k],
                            start=False, stop=last)
            if bank == 0:
                ch.gA = g
            else:
                ch.gB = g

        # ---- phase 2 (final linear) groups per chain pair ----
        movs = {}
        for p, (Xf, Xb) in enumerate(((Xs[0], Xs[1]), (Xs[2], Xs[3]))):
            movs[p] = [X[:, k * XW:k * XW + CHUNK * 128].rearrange(
                           "p (t l) -> p t l", l=128)
                       for X in (Xf, Xb) for k in (0, 1)]
        gstate = {"gi": 0}

        def emit_group(p, p0, glen):
            n = glen * 128
            ps = psA[gstate["gi"] % 2].tile([MEL, 512], F32, tag="g",
                                            name=f"op{p}_{p0}")
            for k in range(4):
                nc.tensor.matmul(ps[:, 0:n], lin_w[:, k * MEL:(k + 1) * MEL],
                                 movs[p][k][:, p0:p0 + glen],
                                 start=(k == 0), stop=(k == 3))
            o_sb = ostage.tile([MEL, 512], F32, tag="os", name=f"os{p}_{p0}")
            nc.vector.tensor_scalar(o_sb[:, 0:n], ps[:, 0:n], lin_b[:], None,
                                    ADD)
            q = (nc.sync, nc.gpsimd)[gstate["gi"] % 2]
            q.dma_start(out_d[:, p, p0:p0 + glen], o_sb[:, 0:n])
            gstate["gi"] += 1

        groups_at = {}
        for p in (0, 1):
            p0 = 0
            while p0 < CHUNK:
                glen = min(4, CHUNK - p0)
                ready = W + max(p0 + glen - 1, CHUNK - 1 - p0)
                groups_at.setdefault(min(ready + (p % 2), K_STEPS - 1),
                                     []).append((p, p0, glen))
                p0 += glen

        def emit_sf_pair(pair, s, ch_a, ch_b):
            sf = actp.tile([128, 1024], BF16, tag=f"sfP{pair}",
                           name=f"sfP{pair}_{s}")
            nc.scalar.activation(sf[:], gpair[(pair, 0)][:], SIG)
            ch_a.sf = sf[:, 0:512]
            ch_b.sf = sf[:, 512:1024]

        def emit_tB_pair(pair, s, ch_a, ch_b):
            t = actp.tile([128, 1024], BF16, tag=f"tBP{pair}",
                          name=f"tBP{pair}_{s}")
            nc.scalar.activation(t[:], gpair[(pair, 1)][:], TANH, scale=0.5)
            ch_a.tB = t[:, 0:512]
            ch_b.tB = t[:, 512:1024]

        def emit_fc(ch, s):
            fc = scr.tile([128, 256], BF16, tag="fc" + ch.name,
                          name=f"fc{ch.name}{s}")
            nc.vector.tensor_mul(fc[:], ch.sf[:, 256:512], ch.c_prev[:])
            ch.fc = fc

        def emit_ig(ch, s):
            ig = scr.tile([128, 256], BF16, tag="ig" + ch.name,
                          name=f"ig{ch.name}{s}")
            nc.vector.tensor_mul(ig[:], ch.sf[:, 0:256], ch.tB[:, 0:256])
            ch.ig = ig

        cpair = {}

        def emit_cnew(ch, s, pair, side):
            # both chains of a pair write one [128,512] c tile so tanh(c)
            # runs as ONE merged act per pair.
            if side == 0:
                cpair[pair] = state.tile([128, 512], BF16, tag=f"cP{pair}",
                                         name=f"cP{pair}_{s}")
            c_new = cpair[pair][:, side * 256:(side + 1) * 256]
            nc.vector.tensor_add(c_new, ch.fc[:], ch.ig[:])
            ch.c_new = c_new

        def emit_tc_pair(pair, s, ch_a, ch_b):
            tc_ = actp.tile([128, 512], BF16, tag=f"tcP{pair}",
                            name=f"tcP{pair}_{s}")
            nc.scalar.activation(tc_[:], cpair[pair][:], TANH)
            ch_a.tc = tc_[:, 0:256]
            ch_b.tc = tc_[:, 256:512]

        def emit_h(ch, s):
            real = s >= W
            t_rel = s - W
            if real:
                lp = t_rel if ch.name[0] == "F" else CHUNK - 1 - t_rel
            else:
                lp = CHUNK + (s & 1)
            dst = tuple(ch.X[:, k * XW + lp * 128:k * XW + (lp + 1) * 128]
                        for k in (0, 1))
            # h2 = (tanh(o/2) + 1) * tanh(c) = 2*sigma(o)*tanh(c); k-half
            # writes so next step's k0 recurrent mms start after half lands.
            for k in (0, 1):
                nc.vector.scalar_tensor_tensor(
                    dst[k], ch.tB[:, 256 + k * 128:256 + (k + 1) * 128], 1.0,
                    ch.tc[:, k * 128:(k + 1) * 128], ADD, MULT)
            ch.src = dst
            ch.c_prev = ch.c_new

        for s in range(K_STEPS):
            emit_dma(s + 1)
            c0, c1, c2, c3 = chains
            # PE order: pair0 bankA (both chains), pair0 bankB, pair1 A, B
            emit_mms(c0, s, 0, 0, 0)
            emit_mms(c1, s, 0, 1, 0)
            emit_mms(c0, s, 0, 0, 1)
            emit_mms(c1, s, 0, 1, 1)
            emit_mms(c2, s, 1, 0, 0)
            emit_mms(c3, s, 1, 1, 0)
            emit_mms(c2, s, 1, 0, 1)
            emit_mms(c3, s, 1, 1, 1)
            emit_dma(s + 2)

            # Act queue: sfP0 tBP0 tcP0 sfP1 tBP1 tcP1 (all pair-merged);
            # DVE trails; h writes follow the pair's tc.
            emit_sf_pair(0, s, c0, c1)
            emit_tB_pair(0, s, c0, c1)
            emit_fc(c0, s)
            emit_ig(c0, s)
            emit_cnew(c0, s, 0, 0)
            emit_fc(c1, s)
            emit_ig(c1, s)
            emit_cnew(c1, s, 0, 1)
            emit_tc_pair(0, s, c0, c1)
            emit_sf_pair(1, s, c2, c3)
            emit_tB_pair(1, s, c2, c3)
            emit_fc(c2, s)
            emit_ig(c2, s)
            emit_cnew(c2, s, 1, 0)
            emit_h(c0, s)
            emit_h(c1, s)
            emit_fc(c3, s)
            emit_ig(c3, s)
            emit_cnew(c3, s, 1, 1)
            emit_tc_pair(1, s, c2, c3)
            emit_h(c2, s)
            emit_h(c3, s)

            for (p, p0g, gl) in groups_at.get(s, []):
                emit_group(p, p0g, gl)

    nc.compile()
    return nc


def _np_lstm_fallback(exp, inputs):
    def sigmoid(z):
        return 1.0 / (1.0 + np.exp(-z))

    def lstm(xs, wih, whh, bih, bhh):
        Bb, L, E = xs.shape
        pre = np.einsum("ble,ge->blg", xs, wih) + bih + bhh
        h = np.zeros((Bb, HID), np.float32)
        c = np.zeros((Bb, HID), np.float32)
        hs = np.zeros((Bb, L, HID), np.float32)
        for t in range(L):
            gg = pre[:, t] + h @ whh.T
            i, f, g_, o = np.split(gg, 4, axis=-1)
            c = sigmoid(f) * c + sigmoid(i) * np.tanh(g_)
            h = sigmoid(o) * np.tanh(c)
            hs[:, t] = h
        return hs

    out_f = lstm(exp, inputs["wih_f"], inputs["whh_f"], inputs["bih_f"],
                 inputs["bhh_f"])
    out_b = lstm(exp[:, ::-1], inputs["wih_b"], inputs["whh_b"],
                 inputs["bih_b"], inputs["bhh_b"])[:, ::-1]
    out = np.concatenate([out_f, out_b], axis=-1)
    return out @ inputs["lin_w"].T + inputs["lin_b"]


def make_in_maps(expP, expR, inputs):
    import ml_dtypes
    bf16 = ml_dtypes.bfloat16
    rows, sc_ih, sc_hh = _mchunk_rows()

    def stat_tiles(w, scale):
        wp = (w.astype(np.float32)[rows] * scale[:, None])
        nk = w.shape[1] // 128
        out = np.zeros((128, 8 * nk * 128), np.float32)
        for m in range(8):
            for k in range(nk):
                out[:, (m * nk + k) * 128:(m * nk + k + 1) * 128] = \
                    wp[m * 128:(m + 1) * 128, k * 128:(k + 1) * 128].T
        return np.ascontiguousarray(out).astype(bf16)

    whhT_f = stat_tiles(inputs["whh_f"], sc_hh)
    whhT_b = stat_tiles(inputs["whh_b"], sc_hh)
    wihT_f = stat_tiles(inputs["wih_f"], sc_ih)
    wihT_b = stat_tiles(inputs["wih_b"], sc_ih)
    lw = inputs["lin_w"].astype(np.float32) * 0.5
    linT = np.concatenate([np.ascontiguousarray(lw[:, k * 128:(k + 1) * 128].T)
                           for k in range(4)], axis=1).astype(bf16)
    lin_b2 = np.ascontiguousarray(inputs["lin_b"].astype(np.float32)[:, None])

    in_maps = []
    for j in range(N_CORES):
        xein = np.zeros((K_STEPS, EMB, 512), np.float32)
        # lane blocks of 64: FA=(4j,4j+1), FB=(4j+2,4j+3),
        # BA=(31-4j,30-4j), BB=(29-4j,28-4j); xe cols FA 0:128, FB 128:256,
        # BA 256:384, BB 384:512
        cks = [4 * j, 4 * j + 1, 4 * j + 2, 4 * j + 3,
               31 - 4 * j, 30 - 4 * j, 29 - 4 * j, 28 - 4 * j]
        srcs = [expP] * 4 + [expR] * 4
        for s in range(K_STEPS):
            for ci, (ck, src) in enumerate(zip(cks, srcs)):
                p = ck * CHUNK - W + s
                if 0 <= p < L_PAD:
                    xein[s, :, ci * 64:(ci + 1) * 64] = src[:, p].T
        in_maps.append({
            "xein": xein.astype(bf16),
            "whhT_f": whhT_f, "whhT_b": whhT_b,
            "wihT_f": wihT_f, "wihT_b": wihT_b,
            "linT": linT, "lin_b": lin_b2,
        })
    return in_maps


def kernel(**inputs):
    global _COMPILED
    inputs = {k: np.asarray(v) for k, v in inputs.items()}
    x = inputs["x"].astype(np.int64)
    exp, L = _host_expand(x, inputs["embed"].astype(np.float32),
                          inputs["dp_w"].astype(np.float32),
                          inputs["dp_b"].astype(np.float32))

    bias_mag = max(float(np.abs(inputs[k]).max())
                   for k in ("bih_f", "bhh_f", "bih_b", "bhh_b"))
    if L > L_PAD or bias_mag != 0.0:
        f32in = {k: (v.astype(np.float32) if v.dtype.kind == "f" else v)
                 for k, v in inputs.items()}
        return _np_lstm_fallback(exp, f32in).astype(np.float32)

    expP = np.zeros((B, L_PAD, EMB), np.float32)
    expP[:, :L] = exp
    expR = expP[:, ::-1]

    in_maps = make_in_maps(expP, expR, inputs)

    if _COMPILED is None:
        _COMPILED = _build_kernel()
    nc = _COMPILED

    res = run_bass_kernel_spmd(nc, in_maps, core_ids=list(range(N_CORES)))

    out = np.empty((B, L_PAD, MEL), np.float32)
    for j in range(N_CORES):
        om = res.results[j]["out_mel"]          # [MEL, 2, CHUNK, 2, B]
        for p in (0, 1):
            for half in (0, 1):
                seg = om[:, p, :, half, :]      # [MEL, CHUNK, B]
                c = 4 * j + 2 * p + half
                out[:, c * CHUNK:(c + 1) * CHUNK] = seg.transpose(2, 1, 0)
    return np.ascontiguousarray(out[:, :L])


if __name__ == "__main__":
    inputs = dict(np.load("/root/problem/inputs.npz"))
    out = kernel(**inputs)
    ref = np.load("/root/problem/expected.npy")
    diff = np.abs(out - ref)
    print("out", out.shape, "absmax diff", diff.max(),
          "rel", diff.max() / np.abs(ref).max())


# revision 32
# speedup vs baseline: 1.1147x; 1.1147x over previous
"""MiniFastSpeech Trainium2 kernel (v6: 4-chain latency-hiding bf16 LSTM).

v3 (2 chains/core) measured loop-bound: the per-step recurrence
dependency chain (mms -> sigmoid -> DVE c-update -> tanh(c) -> h-write ->
mms) is ~4.5us while engine busy is only ~3.2us/step -- ~1.8us/step of
semaphore/pipeline dead time that scheduling cannot remove (every
DVE-produced value costs ~420ns to reach its consumer).

v6 goes busy-bound instead: 4 chains per core (2 fwd + 2 bwd, each 128
lanes = 2 seq-chunks x 64 batch; 32 chunks per direction, CHUNK=21,
W=12 warmup). The period must cover 4 chains' engine work (~6.4us on
Act) which exceeds the ~4.5us chain loop, so the recurrence latency
hides completely. Act work per chain-step: sigmoid [512] over bank A =
[i,f], tanh(x/2) [512] over bank B = [g,o] (g rows pre-doubled -> exact
tanh(g)), tanh [256] of c. DVE (bf16 2x): fc, ig, c_new tensor ops +
2 scalar_tensor_tensor h-writes computing h2 = (tanh(o/2)+1)*tanh(c) =
2h (whh/lin pre-halved on host absorb the 2x). PSUM: 8 banks = 4 chains
x 2 banks, bufs=1; xe matmuls run in-step (no prefetch; PE has slack).
Phase-2 final linear per chain-pair interleaved into the loop; bias add
on Pool.
"""

import sys
import numpy as np
from contextlib import ExitStack

sys.path.insert(0, "/opt/trn_rl_repo")

import concourse.bass as bass
import concourse.tile as tile
from concourse import bacc, mybir
from concourse.bass_utils import run_bass_kernel_spmd

# ---- problem constants (hardcoded per contract) ----
VOCAB, EMB, HID, MEL = 256, 128, 256, 80
B, T = 64, 512
N_CORES = 8
NCHUNK = 32          # chunks per direction
W = 11               # warmup steps per chain
CHUNK = 21           # positions per chunk; L_PAD = 672 >= L
L_PAD = NCHUNK * CHUNK
K_STEPS = W + CHUNK
F32 = mybir.dt.float32
BF16 = mybir.dt.bfloat16
SIG = mybir.ActivationFunctionType.Sigmoid
TANH = mybir.ActivationFunctionType.Tanh
MULT = mybir.AluOpType.mult
ADD = mybir.AluOpType.add

_COMPILED = None


def _host_expand(x, embed, dp_w, dp_b):
    xe = embed[x]                                   # (B,T,E)
    d = np.maximum(xe @ dp_w[0] + dp_b[0], 0)
    dur = np.floor(d).astype(np.int64) + 1
    cum = np.cumsum(dur, axis=1)
    L = int(cum[:, -1].max())
    pos = np.arange(L)
    idx = np.empty((B, L), np.int64)
    for b in range(B):
        idx[b] = np.searchsorted(cum[b], pos, side="right")
    mask = (pos[None, :] < cum[:, -1:]).astype(np.float32)
    exp = np.take_along_axis(xe, np.clip(idx, 0, T - 1)[..., None], axis=1)
    return np.ascontiguousarray(exp * mask[..., None], dtype=np.float32), L


# m-chunk order [i0 i1 f0 f1 | g0 g1 o0 o1]; rows in PyTorch [i,f,g,o] layout.
# Bank A = [i,f] -> one sigmoid act [512]. Bank B = [g,o] -> one tanh(x*0.5)
# act: g rows pre-doubled -> exact tanh(g) in cols 0:256; o gives tanh(o/2).
# whh: additionally all rows halved because the moving h operand is 2h.
def _mchunk_rows():
    rows, sc_ih, sc_hh = [], [], []
    for base, sc in ((0, 1.0), (HID, 1.0), (2 * HID, 2.0), (3 * HID, 1.0)):
        for half in (0, 1):
            rows.append(np.arange(base + half * 128, base + half * 128 + 128))
            sc_ih.append(np.full(128, sc, np.float32))
            sc_hh.append(np.full(128, sc * 0.5, np.float32))
    return (np.concatenate(rows), np.concatenate(sc_ih), np.concatenate(sc_hh))


class _Chain:
    def __init__(self, name, whh, wih, xe_cols, X, poolA, poolB):
        self.name = name
        self.whh = whh          # sbuf [128, 16*128] bf16, tile (m,k) at (2m+k)*128
        self.wih = wih          # sbuf [128, 8*128] bf16, tile m at m*128
        self.xe_cols = xe_cols  # slice in the xein tile
        self.X = X              # sbuf [128, 2*XW] bf16; k-half at k*XW
        self.poolA = poolA
        self.poolB = poolB
        self.gA = None
        self.gB = None
        self.src = None         # (h0, h1) col blocks [128,128] (2h of prev step)
        self.c_prev = None
        self.sf = None
        self.tB = None
        self.fc = None
        self.ig = None
        self.c_new = None
        self.tc = None


def _build_kernel():
    nc = bacc.Bacc("TRN2", target_bir_lowering=False, debug=False,
                   num_devices=N_CORES)

    xein = nc.dram_tensor("xein", [K_STEPS, EMB, 512], BF16,
                          kind="ExternalInput").ap()
    whh_f_d = nc.dram_tensor("whhT_f", [128, 16 * 128], BF16, kind="ExternalInput").ap()
    whh_b_d = nc.dram_tensor("whhT_b", [128, 16 * 128], BF16, kind="ExternalInput").ap()
    wih_f_d = nc.dram_tensor("wihT_f", [128, 8 * 128], BF16, kind="ExternalInput").ap()
    wih_b_d = nc.dram_tensor("wihT_b", [128, 8 * 128], BF16, kind="ExternalInput").ap()
    lin_w_d = nc.dram_tensor("linT", [128, 4 * MEL], BF16, kind="ExternalInput").ap()
    lin_b_d = nc.dram_tensor("lin_b", [MEL, 1], F32, kind="ExternalInput").ap()
    out_d = nc.dram_tensor("out_mel", [MEL, 2, CHUNK, 2, B], F32,
                           kind="ExternalOutput").ap()

    with tile.TileContext(nc) as tc, ExitStack() as ctx:
        wpool = ctx.enter_context(tc.tile_pool(name="weights", bufs=1))
        xpool = ctx.enter_context(tc.tile_pool(name="xstream", bufs=4))
        state = ctx.enter_context(tc.tile_pool(name="state", bufs=3))
        actp = ctx.enter_context(tc.tile_pool(name="acts", bufs=3))
        xbig = ctx.enter_context(tc.tile_pool(name="xbig", bufs=1))
        scr = ctx.enter_context(tc.tile_pool(name="scratch", bufs=3))
        psA = [ctx.enter_context(tc.tile_pool(name=f"gA{i}", bufs=1,
                                              space="PSUM")) for i in range(4)]
        psB = [ctx.enter_context(tc.tile_pool(name=f"gB{i}", bufs=1,
                                              space="PSUM")) for i in range(4)]
        ostage = ctx.enter_context(tc.tile_pool(name="ostage", bufs=2))

        # ---- memsets first (Pool queue) so the PE pre-warm starts at t~0
        hinit = wpool.tile([128, 256], BF16, tag="hinit")
        nc.gpsimd.memset(hinit[:], 0.0)
        zstat_bf = wpool.tile([128, 64], BF16, tag="zstatbf")
        nc.gpsimd.memset(zstat_bf[:], 0.0)

        # PE p-state pre-warm: burn the ramp on dummy matmuls while the
        # weight DMAs are in flight, so step 0 runs at full clock.
        warm = psB[3].tile([128, 512], F32, tag="g", name="pewarm")
        NWARM = 10
        for i in range(NWARM):
            nc.tensor.matmul(warm[0:64, 0:256], zstat_bf[:], hinit[:],
                             start=(i == 0), stop=(i == NWARM - 1))

        # ---- xe stream DMAs ----
        xe_tiles = {}

        def emit_dma(s):
            if s not in xe_tiles and s < K_STEPS:
                xe = xpool.tile([EMB, 512], BF16, tag="xe", name=f"xe{s}")
                nc.sync.dma_start(xe[:], xein[s])
                xe_tiles[s] = xe

        emit_dma(0)
        emit_dma(1)

        # ---- weights -> SBUF
        wih_f = wpool.tile([128, 8 * 128], BF16, tag="wihf")
        nc.scalar.dma_start(wih_f[:], wih_f_d[:])
        wih_b = wpool.tile([128, 8 * 128], BF16, tag="wihb")
        nc.gpsimd.dma_start(wih_b[:], wih_b_d[:])
        whh_f = wpool.tile([128, 16 * 128], BF16, tag="whhf")
        nc.sync.dma_start(whh_f[:], whh_f_d[:])
        whh_b = wpool.tile([128, 16 * 128], BF16, tag="whhb")
        nc.scalar.dma_start(whh_b[:], whh_b_d[:])
        lin_w = wpool.tile([128, 4 * MEL], BF16, tag="linw")
        nc.scalar.dma_start(lin_w[:], lin_w_d[:])
        lin_b = wpool.tile([MEL, 1], F32, tag="linb")
        nc.gpsimd.dma_start(lin_b[:], lin_b_d[:])

        XW = (CHUNK + 2) * 128
        Xs = [xbig.tile([128, 2 * XW], BF16, tag=f"X{i}", name=f"X{i}")
              for i in range(4)]

        # chains: FA, BA, FB, BB (emit order); xe col blocks FA 0:128,
        # FB 128:256, BA 256:384, BB 384:512
        chains = [
            _Chain("FA", whh_f, wih_f, slice(0, 128), Xs[0], psA[0], psB[0]),
            _Chain("BA", whh_b, wih_b, slice(256, 384), Xs[1], psA[1], psB[1]),
            _Chain("FB", whh_f, wih_f, slice(128, 256), Xs[2], psA[2], psB[2]),
            _Chain("BB", whh_b, wih_b, slice(384, 512), Xs[3], psA[3], psB[3]),
        ]
        for ch in chains:
            ch.src = (hinit[:, 0:128], hinit[:, 128:256])
            c0 = state.tile([128, 256], BF16, tag="c" + ch.name,
                            name=f"c0{ch.name}")
            nc.gpsimd.memset(c0[:], 0.0)
            ch.c_prev = c0

        def emit_mms(ch, s):
            # all of bank A (xe + rec, k-major), then bank B. bufs=1: the
            # tile from step s-1 is recycled; Tile waits on its readers.
            xe = xe_tiles[s]
            gA = ch.poolA.tile([128, 512], F32, tag="g", name=f"gA{ch.name}{s}")
            gB = ch.poolB.tile([128, 512], F32, tag="g", name=f"gB{ch.name}{s}")
            for g, mbase in ((gA, 0), (gB, 4)):
                for m in range(mbase, mbase + 4):
                    col = (m % 4) * 128
                    nc.tensor.matmul(g[:, col:col + 128],
                                     ch.wih[:, m * 128:(m + 1) * 128],
                                     xe[:, ch.xe_cols],
                                     start=(m == mbase),
                                     stop=(s == 0 and m == mbase + 3))
                if s > 0:
                    for k in (0, 1):
                        for m in range(mbase, mbase + 4):
                            last = (m == mbase + 3) and (k == 1)
                            col = (m % 4) * 128
                            nc.tensor.matmul(
                                g[:, col:col + 128],
                                ch.whh[:, (2 * m + k) * 128:(2 * m + k + 1) * 128],
                                ch.src[k],
                                start=False, stop=last)
            ch.gA, ch.gB = gA, gB

        # ---- phase 2 (final linear) groups per chain pair ----
        movs = {}
        for p, (Xf, Xb) in enumerate(((Xs[0], Xs[1]), (Xs[2], Xs[3]))):
            movs[p] = [X[:, k * XW:k * XW + CHUNK * 128].rearrange(
                           "p (t l) -> p t l", l=128)
                       for X in (Xf, Xb) for k in (0, 1)]
        gstate = {"gi": 0}

        def emit_group(p, p0, glen):
            n = glen * 128
            ps = psA[gstate["gi"] % 2].tile([MEL, 512], F32, tag="g",
                                            name=f"op{p}_{p0}")
            for k in range(4):
                nc.tensor.matmul(ps[:, 0:n], lin_w[:, k * MEL:(k + 1) * MEL],
                                 movs[p][k][:, p0:p0 + glen],
                                 start=(k == 0), stop=(k == 3))
            o_sb = ostage.tile([MEL, 512], F32, tag="os", name=f"os{p}_{p0}")
            nc.vector.tensor_scalar(o_sb[:, 0:n], ps[:, 0:n], lin_b[:], None,
                                    ADD)
            q = (nc.sync, nc.gpsimd)[gstate["gi"] % 2]
            q.dma_start(out_d[:, p, p0:p0 + glen], o_sb[:, 0:n])
            gstate["gi"] += 1

        groups_at = {}
        for p in (0, 1):
            p0 = 0
            while p0 < CHUNK:
                glen = min(4, CHUNK - p0)
                ready = W + max(p0 + glen - 1, CHUNK - 1 - p0)
                groups_at.setdefault(min(ready + (p % 2), K_STEPS - 1),
                                     []).append((p, p0, glen))
                p0 += glen

        def emit_sf(ch, s):
            sf = actp.tile([128, 512], BF16, tag="sf" + ch.name,
                           name=f"sf{ch.name}{s}")
            nc.scalar.activation(sf[:], ch.gA[:], SIG)
            ch.sf = sf

        def emit_tB(ch, s):
            t = actp.tile([128, 512], BF16, tag="tB" + ch.name,
                          name=f"tB{ch.name}{s}")
            nc.scalar.activation(t[:], ch.gB[:], TANH, scale=0.5)
            ch.tB = t

        def emit_fc(ch, s):
            fc = scr.tile([128, 256], BF16, tag="fc" + ch.name,
                          name=f"fc{ch.name}{s}")
            nc.vector.tensor_mul(fc[:], ch.sf[:, 256:512], ch.c_prev[:])
            ch.fc = fc

        def emit_ig(ch, s):
            ig = scr.tile([128, 256], BF16, tag="ig" + ch.name,
                          name=f"ig{ch.name}{s}")
            nc.vector.tensor_mul(ig[:], ch.sf[:, 0:256], ch.tB[:, 0:256])
            ch.ig = ig

        cpair = {}

        def emit_cnew(ch, s, pair, side):
            # both chains of a pair write one [128,512] c tile so tanh(c)
            # runs as ONE merged act per pair.
            if side == 0:
                cpair[pair] = state.tile([128, 512], BF16, tag=f"cP{pair}",
                                         name=f"cP{pair}_{s}")
            c_new = cpair[pair][:, side * 256:(side + 1) * 256]
            nc.vector.tensor_add(c_new, ch.fc[:], ch.ig[:])
            ch.c_new = c_new

        def emit_tc_pair(pair, s, ch_a, ch_b):
            tc_ = actp.tile([128, 512], BF16, tag=f"tcP{pair}",
                            name=f"tcP{pair}_{s}")
            nc.scalar.activation(tc_[:], cpair[pair][:], TANH)
            ch_a.tc = tc_[:, 0:256]
            ch_b.tc = tc_[:, 256:512]

        def emit_h(ch, s):
            real = s >= W
            t_rel = s - W
            if real:
                lp = t_rel if ch.name[0] == "F" else CHUNK - 1 - t_rel
            else:
                lp = CHUNK + (s & 1)
            dst = tuple(ch.X[:, k * XW + lp * 128:k * XW + (lp + 1) * 128]
                        for k in (0, 1))
            # h2 = (tanh(o/2) + 1) * tanh(c) = 2*sigma(o)*tanh(c); k-half
            # writes so next step's k0 recurrent mms start after half lands.
            for k in (0, 1):
                nc.vector.scalar_tensor_tensor(
                    dst[k], ch.tB[:, 256 + k * 128:256 + (k + 1) * 128], 1.0,
                    ch.tc[:, k * 128:(k + 1) * 128], ADD, MULT)
            ch.src = dst
            ch.c_prev = ch.c_new

        for s in range(K_STEPS):
            emit_dma(s + 1)
            for ch in chains:
                emit_mms(ch, s)
            emit_dma(s + 2)

            # Act queue: sfFA tBFA sfBA tBBA tcP0 sfFB tBFB sfBB tBBB tcP1;
            # DVE trails each chain's acts; h writes follow the pair's tc.
            c0, c1, c2, c3 = chains
            emit_sf(c0, s)
            emit_tB(c0, s)
            emit_sf(c1, s)
            emit_fc(c0, s)
            emit_ig(c0, s)
            emit_cnew(c0, s, 0, 0)
            emit_tB(c1, s)
            emit_fc(c1, s)
            emit_ig(c1, s)
            emit_cnew(c1, s, 0, 1)
            emit_tc_pair(0, s, c0, c1)
            emit_sf(c2, s)
            emit_tB(c2, s)
            emit_fc(c2, s)
            emit_ig(c2, s)
            emit_cnew(c2, s, 1, 0)
            emit_h(c0, s)
            emit_h(c1, s)
            emit_sf(c3, s)
            emit_tB(c3, s)
            emit_fc(c3, s)
            emit_ig(c3, s)
            emit_cnew(c3, s, 1, 1)
            emit_tc_pair(1, s, c2, c3)
            emit_h(c2, s)
            emit_h(c3, s)

            for (p, p0g, gl) in groups_at.get(s, []):
                emit_group(p, p0g, gl)

    nc.compile()
    return nc


def _np_lstm_fallback(exp, inputs):
    def sigmoid(z):
        return 1.0 / (1.0 + np.exp(-z))

    def lstm(xs, wih, whh, bih, bhh):
        Bb, L, E = xs.shape
        pre = np.einsum("ble,ge->blg", xs, wih) + bih + bhh
        h = np.zeros((Bb, HID), np.float32)
        c = np.zeros((Bb, HID), np.float32)
        hs = np.zeros((Bb, L, HID), np.float32)
        for t in range(L):
            gg = pre[:, t] + h @ whh.T
            i, f, g_, o = np.split(gg, 4, axis=-1)
            c = sigmoid(f) * c + sigmoid(i) * np.tanh(g_)
            h = sigmoid(o) * np.tanh(c)
            hs[:, t] = h
        return hs

    out_f = lstm(exp, inputs["wih_f"], inputs["whh_f"], inputs["bih_f"],
                 inputs["bhh_f"])
    out_b = lstm(exp[:, ::-1], inputs["wih_b"], inputs["whh_b"],
                 inputs["bih_b"], inputs["bhh_b"])[:, ::-1]
    out = np.concatenate([out_f, out_b], axis=-1)
    return out @ inputs["lin_w"].T + inputs["lin_b"]


def make_in_maps(expP, expR, inputs):
    import ml_dtypes
    bf16 = ml_dtypes.bfloat16
    rows, sc_ih, sc_hh = _mchunk_rows()

    def stat_tiles(w, scale):
        wp = (w.astype(np.float32)[rows] * scale[:, None])
        nk = w.shape[1] // 128
        out = np.zeros((128, 8 * nk * 128), np.float32)
        for m in range(8):
            for k in range(nk):
                out[:, (m * nk + k) * 128:(m * nk + k + 1) * 128] = \
                    wp[m * 128:(m + 1) * 128, k * 128:(k + 1) * 128].T
        return np.ascontiguousarray(out).astype(bf16)

    whhT_f = stat_tiles(inputs["whh_f"], sc_hh)
    whhT_b = stat_tiles(inputs["whh_b"], sc_hh)
    wihT_f = stat_tiles(inputs["wih_f"], sc_ih)
    wihT_b = stat_tiles(inputs["wih_b"], sc_ih)
    lw = inputs["lin_w"].astype(np.float32) * 0.5
    linT = np.concatenate([np.ascontiguousarray(lw[:, k * 128:(k + 1) * 128].T)
                           for k in range(4)], axis=1).astype(bf16)
    lin_b2 = np.ascontiguousarray(inputs["lin_b"].astype(np.float32)[:, None])

    in_maps = []
    for j in range(N_CORES):
        xein = np.zeros((K_STEPS, EMB, 512), np.float32)
        # lane blocks of 64: FA=(4j,4j+1), FB=(4j+2,4j+3),
        # BA=(31-4j,30-4j), BB=(29-4j,28-4j); xe cols FA 0:128, FB 128:256,
        # BA 256:384, BB 384:512
        cks = [4 * j, 4 * j + 1, 4 * j + 2, 4 * j + 3,
               31 - 4 * j, 30 - 4 * j, 29 - 4 * j, 28 - 4 * j]
        srcs = [expP] * 4 + [expR] * 4
        for s in range(K_STEPS):
            for ci, (ck, src) in enumerate(zip(cks, srcs)):
                p = ck * CHUNK - W + s
                if 0 <= p < L_PAD:
                    xein[s, :, ci * 64:(ci + 1) * 64] = src[:, p].T
        in_maps.append({
            "xein": xein.astype(bf16),
            "whhT_f": whhT_f, "whhT_b": whhT_b,
            "wihT_f": wihT_f, "wihT_b": wihT_b,
            "linT": linT, "lin_b": lin_b2,
        })
    return in_maps


def kernel(**inputs):
    global _COMPILED
    inputs = {k: np.asarray(v) for k, v in inputs.items()}
    x = inputs["x"].astype(np.int64)
    exp, L = _host_expand(x, inputs["embed"].astype(np.float32),
                          inputs["dp_w"].astype(np.float32),
                          inputs["dp_b"].astype(np.float32))

    bias_mag = max(float(np.abs(inputs[k]).max())
                   for k in ("bih_f", "bhh_f", "bih_b", "bhh_b"))
    if L > L_PAD or bias_mag != 0.0:
        f32in = {k: (v.astype(np.float32) if v.dtype.kind == "f" else v)
                 for k, v in inputs.items()}
        return _np_lstm_fallback(exp, f32in).astype(np.float32)

    expP = np.zeros((B, L_PAD, EMB), np.float32)
    expP[:, :L] = exp
    expR = expP[:, ::-1]

    in_maps = make_in_maps(expP, expR, inputs)

    if _COMPILED is None:
        _COMPILED = _build_kernel()
    nc = _COMPILED

    res = run_bass_kernel_spmd(nc, in_maps, core_ids=list(range(N_CORES)))

    out = np.empty((B, L_PAD, MEL), np.float32)
    for j in range(N_CORES):
        om = res.results[j]["out_mel"]          # [MEL, 2, CHUNK, 2, B]
        for p in (0, 1):
            for half in (0, 1):
                seg = om[:, p, :, half, :]      # [MEL, CHUNK, B]
                c = 4 * j + 2 * p + half
                out[:, c * CHUNK:(c + 1) * CHUNK] = seg.transpose(2, 1, 0)
    return np.ascontiguousarray(out[:, :L])


if __name__ == "__main__":
    inputs = dict(np.load("/root/problem/inputs.npz"))
    out = kernel(**inputs)
    ref = np.load("/root/problem/expected.npy")
    diff = np.abs(out - ref)
    print("out", out.shape, "absmax diff", diff.max(),
          "rel", diff.max() / np.abs(ref).max())


# revision 34
# speedup vs baseline: 1.1175x; 1.0025x over previous
"""MiniFastSpeech Trainium2 kernel (v6: 4-chain latency-hiding bf16 LSTM).

v3 (2 chains/core) measured loop-bound: the per-step recurrence
dependency chain (mms -> sigmoid -> DVE c-update -> tanh(c) -> h-write ->
mms) is ~4.5us while engine busy is only ~3.2us/step -- ~1.8us/step of
semaphore/pipeline dead time that scheduling cannot remove (every
DVE-produced value costs ~420ns to reach its consumer).

v6 goes busy-bound instead: 4 chains per core (2 fwd + 2 bwd, each 128
lanes = 2 seq-chunks x 64 batch; 32 chunks per direction, CHUNK=21,
W=12 warmup). The period must cover 4 chains' engine work (~6.4us on
Act) which exceeds the ~4.5us chain loop, so the recurrence latency
hides completely. Act work per chain-step: sigmoid [512] over bank A =
[i,f], tanh(x/2) [512] over bank B = [g,o] (g rows pre-doubled -> exact
tanh(g)), tanh [256] of c. DVE (bf16 2x): fc, ig, c_new tensor ops +
2 scalar_tensor_tensor h-writes computing h2 = (tanh(o/2)+1)*tanh(c) =
2h (whh/lin pre-halved on host absorb the 2x). PSUM: 8 banks = 4 chains
x 2 banks, bufs=1; xe matmuls run in-step (no prefetch; PE has slack).
Phase-2 final linear per chain-pair interleaved into the loop; bias add
on Pool.
"""

import sys
import numpy as np
from contextlib import ExitStack

sys.path.insert(0, "/opt/trn_rl_repo")

import concourse.bass as bass
import concourse.tile as tile
from concourse import bacc, mybir
from concourse.bass_utils import run_bass_kernel_spmd

# ---- problem constants (hardcoded per contract) ----
VOCAB, EMB, HID, MEL = 256, 128, 256, 80
B, T = 64, 512
N_CORES = 8
NCHUNK = 32          # chunks per direction
W = 11               # warmup steps per chain
CHUNK = 21           # positions per chunk; L_PAD = 672 >= L
L_PAD = NCHUNK * CHUNK
K_STEPS = W + CHUNK
F32 = mybir.dt.float32
BF16 = mybir.dt.bfloat16
SIG = mybir.ActivationFunctionType.Sigmoid
TANH = mybir.ActivationFunctionType.Tanh
MULT = mybir.AluOpType.mult
ADD = mybir.AluOpType.add

_COMPILED = None


def _host_expand(x, embed, dp_w, dp_b):
    xe = embed[x]                                   # (B,T,E)
    d = np.maximum(xe @ dp_w[0] + dp_b[0], 0)
    dur = np.floor(d).astype(np.int64) + 1
    cum = np.cumsum(dur, axis=1)
    L = int(cum[:, -1].max())
    pos = np.arange(L)
    idx = np.empty((B, L), np.int64)
    for b in range(B):
        idx[b] = np.searchsorted(cum[b], pos, side="right")
    mask = (pos[None, :] < cum[:, -1:]).astype(np.float32)
    exp = np.take_along_axis(xe, np.clip(idx, 0, T - 1)[..., None], axis=1)
    return np.ascontiguousarray(exp * mask[..., None], dtype=np.float32), L


# m-chunk order [i0 i1 f0 f1 | g0 g1 o0 o1]; rows in PyTorch [i,f,g,o] layout.
# Bank A = [i,f] -> one sigmoid act [512]. Bank B = [g,o] -> one tanh(x*0.5)
# act: g rows pre-doubled -> exact tanh(g) in cols 0:256; o gives tanh(o/2).
# whh: additionally all rows halved because the moving h operand is 2h.
def _mchunk_rows():
    rows, sc_ih, sc_hh = [], [], []
    for base, sc in ((0, 1.0), (HID, 1.0), (2 * HID, 2.0), (3 * HID, 1.0)):
        for half in (0, 1):
            rows.append(np.arange(base + half * 128, base + half * 128 + 128))
            sc_ih.append(np.full(128, sc, np.float32))
            sc_hh.append(np.full(128, sc * 0.5, np.float32))
    return (np.concatenate(rows), np.concatenate(sc_ih), np.concatenate(sc_hh))


class _Chain:
    def __init__(self, name, whh, wih, xe_cols, X, poolA, poolB):
        self.name = name
        self.whh = whh          # sbuf [128, 16*128] bf16, tile (m,k) at (2m+k)*128
        self.wih = wih          # sbuf [128, 8*128] bf16, tile m at m*128
        self.xe_cols = xe_cols  # slice in the xein tile
        self.X = X              # sbuf [128, 2*XW] bf16; k-half at k*XW
        self.poolA = poolA
        self.poolB = poolB
        self.gA = None
        self.gB = None
        self.src = None         # (h0, h1) col blocks [128,128] (2h of prev step)
        self.c_prev = None
        self.sf = None
        self.tB = None
        self.fc = None
        self.ig = None
        self.c_new = None
        self.tc = None


def _build_kernel():
    nc = bacc.Bacc("TRN2", target_bir_lowering=False, debug=False,
                   num_devices=N_CORES)

    xein = nc.dram_tensor("xein", [K_STEPS, EMB, 512], BF16,
                          kind="ExternalInput").ap()
    whh_f_d = nc.dram_tensor("whhT_f", [128, 16 * 128], BF16, kind="ExternalInput").ap()
    whh_b_d = nc.dram_tensor("whhT_b", [128, 16 * 128], BF16, kind="ExternalInput").ap()
    wih_f_d = nc.dram_tensor("wihT_f", [128, 8 * 128], BF16, kind="ExternalInput").ap()
    wih_b_d = nc.dram_tensor("wihT_b", [128, 8 * 128], BF16, kind="ExternalInput").ap()
    lin_w_d = nc.dram_tensor("linT", [128, 4 * MEL], BF16, kind="ExternalInput").ap()
    lin_b_d = nc.dram_tensor("lin_b", [MEL, 1], F32, kind="ExternalInput").ap()
    out_d = nc.dram_tensor("out_mel", [MEL, 2, CHUNK, 2, B], F32,
                           kind="ExternalOutput").ap()

    with tile.TileContext(nc) as tc, ExitStack() as ctx:
        wpool = ctx.enter_context(tc.tile_pool(name="weights", bufs=1))
        xpool = ctx.enter_context(tc.tile_pool(name="xstream", bufs=4))
        state = ctx.enter_context(tc.tile_pool(name="state", bufs=3))
        actp = ctx.enter_context(tc.tile_pool(name="acts", bufs=3))
        xbig = ctx.enter_context(tc.tile_pool(name="xbig", bufs=1))
        scr = ctx.enter_context(tc.tile_pool(name="scratch", bufs=3))
        psA = [ctx.enter_context(tc.tile_pool(name=f"gA{i}", bufs=1,
                                              space="PSUM")) for i in range(4)]
        psB = [ctx.enter_context(tc.tile_pool(name=f"gB{i}", bufs=1,
                                              space="PSUM")) for i in range(4)]
        ostage = ctx.enter_context(tc.tile_pool(name="ostage", bufs=2))

        # ---- memsets first (Pool queue) so the PE pre-warm starts at t~0
        hinit = wpool.tile([128, 256], BF16, tag="hinit")
        nc.gpsimd.memset(hinit[:], 0.0)
        zstat_bf = wpool.tile([128, 64], BF16, tag="zstatbf")
        nc.gpsimd.memset(zstat_bf[:], 0.0)

        # PE p-state pre-warm: burn the ramp on dummy matmuls while the
        # weight DMAs are in flight, so step 0 runs at full clock.
        warm = psB[3].tile([128, 512], F32, tag="g", name="pewarm")
        NWARM = 10
        for i in range(NWARM):
            nc.tensor.matmul(warm[0:64, 0:256], zstat_bf[:], hinit[:],
                             start=(i == 0), stop=(i == NWARM - 1))

        # ---- xe stream DMAs ----
        xe_tiles = {}

        def emit_dma(s):
            if s not in xe_tiles and s < K_STEPS:
                xe = xpool.tile([EMB, 512], BF16, tag="xe", name=f"xe{s}")
                nc.sync.dma_start(xe[:], xein[s])
                xe_tiles[s] = xe

        emit_dma(0)
        emit_dma(1)

        # ---- weights -> SBUF
        wih_f = wpool.tile([128, 8 * 128], BF16, tag="wihf")
        nc.scalar.dma_start(wih_f[:], wih_f_d[:])
        wih_b = wpool.tile([128, 8 * 128], BF16, tag="wihb")
        nc.gpsimd.dma_start(wih_b[:], wih_b_d[:])
        whh_f = wpool.tile([128, 16 * 128], BF16, tag="whhf")
        nc.sync.dma_start(whh_f[:], whh_f_d[:])
        whh_b = wpool.tile([128, 16 * 128], BF16, tag="whhb")
        nc.scalar.dma_start(whh_b[:], whh_b_d[:])
        lin_w = wpool.tile([128, 4 * MEL], BF16, tag="linw")
        nc.scalar.dma_start(lin_w[:], lin_w_d[:])
        lin_b = wpool.tile([MEL, 1], F32, tag="linb")
        nc.gpsimd.dma_start(lin_b[:], lin_b_d[:])

        XW = (CHUNK + 2) * 128
        Xs = [xbig.tile([128, 2 * XW], BF16, tag=f"X{i}", name=f"X{i}")
              for i in range(4)]

        # chains: FA, BA, FB, BB (emit order); xe col blocks FA 0:128,
        # FB 128:256, BA 256:384, BB 384:512
        chains = [
            _Chain("FA", whh_f, wih_f, slice(0, 128), Xs[0], psA[0], psB[0]),
            _Chain("BA", whh_b, wih_b, slice(256, 384), Xs[1], psA[1], psB[1]),
            _Chain("FB", whh_f, wih_f, slice(128, 256), Xs[2], psA[2], psB[2]),
            _Chain("BB", whh_b, wih_b, slice(384, 512), Xs[3], psA[3], psB[3]),
        ]
        for ch in chains:
            ch.src = (hinit[:, 0:128], hinit[:, 128:256])
            c0 = state.tile([128, 256], BF16, tag="c" + ch.name,
                            name=f"c0{ch.name}")
            nc.gpsimd.memset(c0[:], 0.0)
            ch.c_prev = c0

        def emit_mms(ch, s):
            # all of bank A (xe + rec, k-major), then bank B. bufs=1: the
            # tile from step s-1 is recycled; Tile waits on its readers.
            xe = xe_tiles[s]
            gA = ch.poolA.tile([128, 512], F32, tag="g", name=f"gA{ch.name}{s}")
            gB = ch.poolB.tile([128, 512], F32, tag="g", name=f"gB{ch.name}{s}")
            for g, mbase in ((gA, 0), (gB, 4)):
                for m in range(mbase, mbase + 4):
                    col = (m % 4) * 128
                    nc.tensor.matmul(g[:, col:col + 128],
                                     ch.wih[:, m * 128:(m + 1) * 128],
                                     xe[:, ch.xe_cols],
                                     start=(m == mbase),
                                     stop=(s == 0 and m == mbase + 3))
                if s > 0:
                    for k in (0, 1):
                        for m in range(mbase, mbase + 4):
                            last = (m == mbase + 3) and (k == 1)
                            col = (m % 4) * 128
                            nc.tensor.matmul(
                                g[:, col:col + 128],
                                ch.whh[:, (2 * m + k) * 128:(2 * m + k + 1) * 128],
                                ch.src[k],
                                start=False, stop=last)
            ch.gA, ch.gB = gA, gB
            ch.gA_tile = gA

        # ---- phase 2 (final linear) groups per chain pair ----
        movs = {}
        for p, (Xf, Xb) in enumerate(((Xs[0], Xs[1]), (Xs[2], Xs[3]))):
            movs[p] = [X[:, k * XW:k * XW + CHUNK * 128].rearrange(
                           "p (t l) -> p t l", l=128)
                       for X in (Xf, Xb) for k in (0, 1)]
        gstate = {"gi": 0}

        def emit_group(p, p0, glen):
            n = glen * 128
            # Reuse the current step's (already activated) gate tile of a
            # rotating chain as the phase-2 PSUM accumulator: no pool
            # cycling, so the next step's matmuls never stall on the
            # bias-add. start=True re-zeroes the region (sigma already
            # consumed it).
            ps = chains[gstate["gi"] % 4].gA_tile[0:MEL, :]
            for k in range(4):
                nc.tensor.matmul(ps[:, 0:n], lin_w[:, k * MEL:(k + 1) * MEL],
                                 movs[p][k][:, p0:p0 + glen],
                                 start=(k == 0), stop=(k == 3))
            o_sb = ostage.tile([MEL, 512], F32, tag="os", name=f"os{p}_{p0}")
            nc.vector.tensor_scalar(o_sb[:, 0:n], ps[:, 0:n], lin_b[:], None,
                                    ADD)
            q = (nc.sync, nc.gpsimd)[gstate["gi"] % 2]
            q.dma_start(out_d[:, p, p0:p0 + glen], o_sb[:, 0:n])
            gstate["gi"] += 1

        groups_at = {}
        for p in (0, 1):
            p0 = 0
            while p0 < CHUNK:
                glen = min(4, CHUNK - p0)
                ready = W + max(p0 + glen - 1, CHUNK - 1 - p0)
                groups_at.setdefault(min(ready + (p % 2), K_STEPS - 1),
                                     []).append((p, p0, glen))
                p0 += glen

        def emit_sf(ch, s):
            sf = actp.tile([128, 512], BF16, tag="sf" + ch.name,
                           name=f"sf{ch.name}{s}")
            nc.scalar.activation(sf[:], ch.gA[:], SIG)
            ch.sf = sf

        def emit_tB(ch, s):
            t = actp.tile([128, 512], BF16, tag="tB" + ch.name,
                          name=f"tB{ch.name}{s}")
            nc.scalar.activation(t[:], ch.gB[:], TANH, scale=0.5)
            ch.tB = t

        def emit_fc(ch, s):
            fc = scr.tile([128, 256], BF16, tag="fc" + ch.name,
                          name=f"fc{ch.name}{s}")
            nc.vector.tensor_mul(fc[:], ch.sf[:, 256:512], ch.c_prev[:])
            ch.fc = fc

        def emit_ig(ch, s):
            ig = scr.tile([128, 256], BF16, tag="ig" + ch.name,
                          name=f"ig{ch.name}{s}")
            nc.vector.tensor_mul(ig[:], ch.sf[:, 0:256], ch.tB[:, 0:256])
            ch.ig = ig

        cpair = {}

        def emit_cnew(ch, s, pair, side):
            # both chains of a pair write one [128,512] c tile so tanh(c)
            # runs as ONE merged act per pair.
            if side == 0:
                cpair[pair] = state.tile([128, 512], BF16, tag=f"cP{pair}",
                                         name=f"cP{pair}_{s}")
            c_new = cpair[pair][:, side * 256:(side + 1) * 256]
            nc.vector.tensor_add(c_new, ch.fc[:], ch.ig[:])
            ch.c_new = c_new

        def emit_tc_pair(pair, s, ch_a, ch_b):
            tc_ = actp.tile([128, 512], BF16, tag=f"tcP{pair}",
                            name=f"tcP{pair}_{s}")
            nc.scalar.activation(tc_[:], cpair[pair][:], TANH)
            ch_a.tc = tc_[:, 0:256]
            ch_b.tc = tc_[:, 256:512]

        def emit_h(ch, s):
            real = s >= W
            t_rel = s - W
            if real:
                lp = t_rel if ch.name[0] == "F" else CHUNK - 1 - t_rel
            else:
                lp = CHUNK + (s & 1)
            dst = tuple(ch.X[:, k * XW + lp * 128:k * XW + (lp + 1) * 128]
                        for k in (0, 1))
            # h2 = (tanh(o/2) + 1) * tanh(c) = 2*sigma(o)*tanh(c); k-half
            # writes so next step's k0 recurrent mms start after half lands.
            for k in (0, 1):
                nc.vector.scalar_tensor_tensor(
                    dst[k], ch.tB[:, 256 + k * 128:256 + (k + 1) * 128], 1.0,
                    ch.tc[:, k * 128:(k + 1) * 128], ADD, MULT)
            ch.src = dst
            ch.c_prev = ch.c_new

        for s in range(K_STEPS):
            emit_dma(s + 1)
            for ch in chains:
                emit_mms(ch, s)
            emit_dma(s + 2)

            # Act queue: sfFA tBFA sfBA tBBA tcP0 sfFB tBFB sfBB tBBB tcP1;
            # DVE trails each chain's acts; h writes follow the pair's tc.
            c0, c1, c2, c3 = chains
            emit_sf(c0, s)
            emit_tB(c0, s)
            emit_sf(c1, s)
            emit_fc(c0, s)
            emit_ig(c0, s)
            emit_cnew(c0, s, 0, 0)
            emit_tB(c1, s)
            emit_fc(c1, s)
            emit_ig(c1, s)
            emit_cnew(c1, s, 0, 1)
            emit_tc_pair(0, s, c0, c1)
            emit_sf(c2, s)
            emit_tB(c2, s)
            emit_fc(c2, s)
            emit_ig(c2, s)
            emit_cnew(c2, s, 1, 0)
            emit_h(c0, s)
            emit_h(c1, s)
            emit_sf(c3, s)
            emit_tB(c3, s)
            emit_fc(c3, s)
            emit_ig(c3, s)
            emit_cnew(c3, s, 1, 1)
            emit_tc_pair(1, s, c2, c3)
            emit_h(c2, s)
            emit_h(c3, s)

            for (p, p0g, gl) in groups_at.get(s, []):
                emit_group(p, p0g, gl)

    nc.compile()
    return nc


def _np_lstm_fallback(exp, inputs):
    def sigmoid(z):
        return 1.0 / (1.0 + np.exp(-z))

    def lstm(xs, wih, whh, bih, bhh):
        Bb, L, E = xs.shape
        pre = np.einsum("ble,ge->blg", xs, wih) + bih + bhh
        h = np.zeros((Bb, HID), np.float32)
        c = np.zeros((Bb, HID), np.float32)
        hs = np.zeros((Bb, L, HID), np.float32)
        for t in range(L):
            gg = pre[:, t] + h @ whh.T
            i, f, g_, o = np.split(gg, 4, axis=-1)
            c = sigmoid(f) * c + sigmoid(i) * np.tanh(g_)
            h = sigmoid(o) * np.tanh(c)
            hs[:, t] = h
        return hs

    out_f = lstm(exp, inputs["wih_f"], inputs["whh_f"], inputs["bih_f"],
                 inputs["bhh_f"])
    out_b = lstm(exp[:, ::-1], inputs["wih_b"], inputs["whh_b"],
                 inputs["bih_b"], inputs["bhh_b"])[:, ::-1]
    out = np.concatenate([out_f, out_b], axis=-1)
    return out @ inputs["lin_w"].T + inputs["lin_b"]


def make_in_maps(expP, expR, inputs):
    import ml_dtypes
    bf16 = ml_dtypes.bfloat16
    rows, sc_ih, sc_hh = _mchunk_rows()

    def stat_tiles(w, scale):
        wp = (w.astype(np.float32)[rows] * scale[:, None])
        nk = w.shape[1] // 128
        out = np.zeros((128, 8 * nk * 128), np.float32)
        for m in range(8):
            for k in range(nk):
                out[:, (m * nk + k) * 128:(m * nk + k + 1) * 128] = \
                    wp[m * 128:(m + 1) * 128, k * 128:(k + 1) * 128].T
        return np.ascontiguousarray(out).astype(bf16)

    whhT_f = stat_tiles(inputs["whh_f"], sc_hh)
    whhT_b = stat_tiles(inputs["whh_b"], sc_hh)
    wihT_f = stat_tiles(inputs["wih_f"], sc_ih)
    wihT_b = stat_tiles(inputs["wih_b"], sc_ih)
    lw = inputs["lin_w"].astype(np.float32) * 0.5
    linT = np.concatenate([np.ascontiguousarray(lw[:, k * 128:(k + 1) * 128].T)
                           for k in range(4)], axis=1).astype(bf16)
    lin_b2 = np.ascontiguousarray(inputs["lin_b"].astype(np.float32)[:, None])

    in_maps = []
    for j in range(N_CORES):
        xein = np.zeros((K_STEPS, EMB, 512), np.float32)
        # lane blocks of 64: FA=(4j,4j+1), FB=(4j+2,4j+3),
        # BA=(31-4j,30-4j), BB=(29-4j,28-4j); xe cols FA 0:128, FB 128:256,
        # BA 256:384, BB 384:512
        cks = [4 * j, 4 * j + 1, 4 * j + 2, 4 * j + 3,
               31 - 4 * j, 30 - 4 * j, 29 - 4 * j, 28 - 4 * j]
        srcs = [expP] * 4 + [expR] * 4
        for s in range(K_STEPS):
            for ci, (ck, src) in enumerate(zip(cks, srcs)):
                p = ck * CHUNK - W + s
                if 0 <= p < L_PAD:
                    xein[s, :, ci * 64:(ci + 1) * 64] = src[:, p].T
        in_maps.append({
            "xein": xein.astype(bf16),
            "whhT_f": whhT_f, "whhT_b": whhT_b,
            "wihT_f": wihT_f, "wihT_b": wihT_b,
            "linT": linT, "lin_b": lin_b2,
        })
    return in_maps


def kernel(**inputs):
    global _COMPILED
    inputs = {k: np.asarray(v) for k, v in inputs.items()}
    x = inputs["x"].astype(np.int64)
    exp, L = _host_expand(x, inputs["embed"].astype(np.float32),
                          inputs["dp_w"].astype(np.float32),
                          inputs["dp_b"].astype(np.float32))

    bias_mag = max(float(np.abs(inputs[k]).max())
                   for k in ("bih_f", "bhh_f", "bih_b", "bhh_b"))
    if L > L_PAD or bias_mag != 0.0:
        f32in = {k: (v.astype(np.float32) if v.dtype.kind == "f" else v)
                 for k, v in inputs.items()}
        return _np_lstm_fallback(exp, f32in).astype(np.float32)

    expP = np.zeros((B, L_PAD, EMB), np.float32)
    expP[:, :L] = exp
    expR = expP[:, ::-1]

    in_maps = make_in_maps(expP, expR, inputs)

    if _COMPILED is None:
        _COMPILED = _build_kernel()
    nc = _COMPILED

    res = run_bass_kernel_spmd(nc, in_maps, core_ids=list(range(N_CORES)))

    out = np.empty((B, L_PAD, MEL), np.float32)
    for j in range(N_CORES):
        om = res.results[j]["out_mel"]          # [MEL, 2, CHUNK, 2, B]
        for p in (0, 1):
            for half in (0, 1):
                seg = om[:, p, :, half, :]      # [MEL, CHUNK, B]
                c = 4 * j + 2 * p + half
                out[:, c * CHUNK:(c + 1) * CHUNK] = seg.transpose(2, 1, 0)
    return np.ascontiguousarray(out[:, :L])


if __name__ == "__main__":
    inputs = dict(np.load("/root/problem/inputs.npz"))
    out = kernel(**inputs)
    ref = np.load("/root/problem/expected.npy")
    diff = np.abs(out - ref)
    print("out", out.shape, "absmax diff", diff.max(),
          "rel", diff.max() / np.abs(ref).max())


# revision 35
# speedup vs baseline: 1.1195x; 1.0018x over previous
"""MiniFastSpeech Trainium2 kernel (v6: 4-chain latency-hiding bf16 LSTM).

v3 (2 chains/core) measured loop-bound: the per-step recurrence
dependency chain (mms -> sigmoid -> DVE c-update -> tanh(c) -> h-write ->
mms) is ~4.5us while engine busy is only ~3.2us/step -- ~1.8us/step of
semaphore/pipeline dead time that scheduling cannot remove (every
DVE-produced value costs ~420ns to reach its consumer).

v6 goes busy-bound instead: 4 chains per core (2 fwd + 2 bwd, each 128
lanes = 2 seq-chunks x 64 batch; 32 chunks per direction, CHUNK=21,
W=12 warmup). The period must cover 4 chains' engine work (~6.4us on
Act) which exceeds the ~4.5us chain loop, so the recurrence latency
hides completely. Act work per chain-step: sigmoid [512] over bank A =
[i,f], tanh(x/2) [512] over bank B = [g,o] (g rows pre-doubled -> exact
tanh(g)), tanh [256] of c. DVE (bf16 2x): fc, ig, c_new tensor ops +
2 scalar_tensor_tensor h-writes computing h2 = (tanh(o/2)+1)*tanh(c) =
2h (whh/lin pre-halved on host absorb the 2x). PSUM: 8 banks = 4 chains
x 2 banks, bufs=1; xe matmuls run in-step (no prefetch; PE has slack).
Phase-2 final linear per chain-pair interleaved into the loop; bias add
on Pool.
"""

import sys
import numpy as np
from contextlib import ExitStack

sys.path.insert(0, "/opt/trn_rl_repo")

import concourse.bass as bass
import concourse.tile as tile
from concourse import bacc, mybir
from concourse.bass_utils import run_bass_kernel_spmd

# ---- problem constants (hardcoded per contract) ----
VOCAB, EMB, HID, MEL = 256, 128, 256, 80
B, T = 64, 512
N_CORES = 8
NCHUNK = 32          # chunks per direction
W = 11               # warmup steps per chain
CHUNK = 21           # positions per chunk; L_PAD = 672 >= L
L_PAD = NCHUNK * CHUNK
K_STEPS = W + CHUNK
F32 = mybir.dt.float32
BF16 = mybir.dt.bfloat16
SIG = mybir.ActivationFunctionType.Sigmoid
TANH = mybir.ActivationFunctionType.Tanh
MULT = mybir.AluOpType.mult
ADD = mybir.AluOpType.add

_COMPILED = None


def _host_expand(x, embed, dp_w, dp_b):
    xe = embed[x]                                   # (B,T,E)
    d = np.maximum(xe @ dp_w[0] + dp_b[0], 0)
    dur = np.floor(d).astype(np.int64) + 1
    cum = np.cumsum(dur, axis=1)
    L = int(cum[:, -1].max())
    pos = np.arange(L)
    idx = np.empty((B, L), np.int64)
    for b in range(B):
        idx[b] = np.searchsorted(cum[b], pos, side="right")
    mask = (pos[None, :] < cum[:, -1:]).astype(np.float32)
    exp = np.take_along_axis(xe, np.clip(idx, 0, T - 1)[..., None], axis=1)
    return np.ascontiguousarray(exp * mask[..., None], dtype=np.float32), L


# m-chunk order [i0 i1 f0 f1 | g0 g1 o0 o1]; rows in PyTorch [i,f,g,o] layout.
# Bank A = [i,f] -> one sigmoid act [512]. Bank B = [g,o] -> one tanh(x*0.5)
# act: g rows pre-doubled -> exact tanh(g) in cols 0:256; o gives tanh(o/2).
# whh: additionally all rows halved because the moving h operand is 2h.
def _mchunk_rows():
    rows, sc_ih, sc_hh = [], [], []
    for base, sc in ((0, 1.0), (HID, 1.0), (2 * HID, 2.0), (3 * HID, 1.0)):
        for half in (0, 1):
            rows.append(np.arange(base + half * 128, base + half * 128 + 128))
            sc_ih.append(np.full(128, sc, np.float32))
            sc_hh.append(np.full(128, sc * 0.5, np.float32))
    return (np.concatenate(rows), np.concatenate(sc_ih), np.concatenate(sc_hh))


class _Chain:
    def __init__(self, name, whh, wih, xe_cols, X, poolA, poolB):
        self.name = name
        self.whh = whh          # sbuf [128, 16*128] bf16, tile (m,k) at (2m+k)*128
        self.wih = wih          # sbuf [128, 8*128] bf16, tile m at m*128
        self.xe_cols = xe_cols  # slice in the xein tile
        self.X = X              # sbuf [128, 2*XW] bf16; k-half at k*XW
        self.poolA = poolA
        self.poolB = poolB
        self.gA = None
        self.gB = None
        self.src = None         # (h0, h1) col blocks [128,128] (2h of prev step)
        self.c_prev = None
        self.sf = None
        self.tB = None
        self.fc = None
        self.ig = None
        self.c_new = None
        self.tc = None


def _build_kernel():
    nc = bacc.Bacc("TRN2", target_bir_lowering=False, debug=False,
                   num_devices=N_CORES)

    xein = nc.dram_tensor("xein", [K_STEPS, EMB, 512], BF16,
                          kind="ExternalInput").ap()
    whh_f_d = nc.dram_tensor("whhT_f", [128, 16 * 128], BF16, kind="ExternalInput").ap()
    whh_b_d = nc.dram_tensor("whhT_b", [128, 16 * 128], BF16, kind="ExternalInput").ap()
    wih_f_d = nc.dram_tensor("wihT_f", [128, 8 * 128], BF16, kind="ExternalInput").ap()
    wih_b_d = nc.dram_tensor("wihT_b", [128, 8 * 128], BF16, kind="ExternalInput").ap()
    lin_w_d = nc.dram_tensor("linT", [128, 4 * MEL], BF16, kind="ExternalInput").ap()
    lin_b_d = nc.dram_tensor("lin_b", [MEL, 1], F32, kind="ExternalInput").ap()
    out_d = nc.dram_tensor("out_mel", [MEL, 2, CHUNK, 2, B], F32,
                           kind="ExternalOutput").ap()

    with tile.TileContext(nc) as tc, ExitStack() as ctx:
        wpool = ctx.enter_context(tc.tile_pool(name="weights", bufs=1))
        xpool = ctx.enter_context(tc.tile_pool(name="xstream", bufs=4))
        state = ctx.enter_context(tc.tile_pool(name="state", bufs=3))
        actp = ctx.enter_context(tc.tile_pool(name="acts", bufs=3))
        xbig = ctx.enter_context(tc.tile_pool(name="xbig", bufs=1))
        scr = ctx.enter_context(tc.tile_pool(name="scratch", bufs=3))
        psA = [ctx.enter_context(tc.tile_pool(name=f"gA{i}", bufs=1,
                                              space="PSUM")) for i in range(4)]
        psB = [ctx.enter_context(tc.tile_pool(name=f"gB{i}", bufs=1,
                                              space="PSUM")) for i in range(4)]
        ostage = ctx.enter_context(tc.tile_pool(name="ostage", bufs=2))

        # ---- memsets first (Pool queue) so the PE pre-warm starts at t~0
        hinit = wpool.tile([128, 256], BF16, tag="hinit")
        nc.gpsimd.memset(hinit[:], 0.0)
        zstat_bf = wpool.tile([128, 64], BF16, tag="zstatbf")
        nc.gpsimd.memset(zstat_bf[:], 0.0)

        # PE p-state pre-warm: burn the ramp on dummy matmuls while the
        # weight DMAs are in flight, so step 0 runs at full clock.
        warm = psB[3].tile([128, 512], F32, tag="g", name="pewarm")
        NWARM = 10
        for i in range(NWARM):
            nc.tensor.matmul(warm[0:64, 0:256], zstat_bf[:], hinit[:],
                             start=(i == 0), stop=(i == NWARM - 1))

        # ---- xe stream DMAs ----
        xe_tiles = {}

        def emit_dma(s):
            if s not in xe_tiles and s < K_STEPS:
                xe = xpool.tile([EMB, 512], BF16, tag="xe", name=f"xe{s}")
                nc.sync.dma_start(xe[:], xein[s])
                xe_tiles[s] = xe

        emit_dma(0)
        emit_dma(1)

        # ---- weights -> SBUF
        wih_f = wpool.tile([128, 8 * 128], BF16, tag="wihf")
        nc.scalar.dma_start(wih_f[:], wih_f_d[:])
        wih_b = wpool.tile([128, 8 * 128], BF16, tag="wihb")
        nc.gpsimd.dma_start(wih_b[:], wih_b_d[:])
        whh_f = wpool.tile([128, 16 * 128], BF16, tag="whhf")
        nc.sync.dma_start(whh_f[:], whh_f_d[:])
        whh_b = wpool.tile([128, 16 * 128], BF16, tag="whhb")
        nc.scalar.dma_start(whh_b[:], whh_b_d[:])
        lin_w = wpool.tile([128, 4 * MEL], BF16, tag="linw")
        nc.scalar.dma_start(lin_w[:], lin_w_d[:])
        lin_b = wpool.tile([MEL, 1], F32, tag="linb")
        nc.gpsimd.dma_start(lin_b[:], lin_b_d[:])

        XW = (CHUNK + 2) * 128
        Xs = [xbig.tile([128, 2 * XW], BF16, tag=f"X{i}", name=f"X{i}")
              for i in range(4)]

        # chains: FA, BA, FB, BB (emit order); xe col blocks FA 0:128,
        # FB 128:256, BA 256:384, BB 384:512
        chains = [
            _Chain("FA", whh_f, wih_f, slice(0, 128), Xs[0], psA[0], psB[0]),
            _Chain("BA", whh_b, wih_b, slice(256, 384), Xs[1], psA[1], psB[1]),
            _Chain("FB", whh_f, wih_f, slice(128, 256), Xs[2], psA[2], psB[2]),
            _Chain("BB", whh_b, wih_b, slice(384, 512), Xs[3], psA[3], psB[3]),
        ]
        for ch in chains:
            ch.src = (hinit[:, 0:128], hinit[:, 128:256])
            c0 = state.tile([128, 256], BF16, tag="c" + ch.name,
                            name=f"c0{ch.name}")
            nc.gpsimd.memset(c0[:], 0.0)
            ch.c_prev = c0

        def emit_mms(ch, s):
            # all of bank A (xe + rec, k-major), then bank B. bufs=1: the
            # tile from step s-1 is recycled; Tile waits on its readers.
            xe = xe_tiles[s]
            gA = ch.poolA.tile([128, 512], F32, tag="g", name=f"gA{ch.name}{s}")
            gB = ch.poolB.tile([128, 512], F32, tag="g", name=f"gB{ch.name}{s}")
            for g, mbase in ((gA, 0), (gB, 4)):
                for m in range(mbase, mbase + 4):
                    col = (m % 4) * 128
                    nc.tensor.matmul(g[:, col:col + 128],
                                     ch.wih[:, m * 128:(m + 1) * 128],
                                     xe[:, ch.xe_cols],
                                     start=(m == mbase),
                                     stop=(s == 0 and m == mbase + 3))
                if s > 0:
                    for k in (0, 1):
                        for m in range(mbase, mbase + 4):
                            last = (m == mbase + 3) and (k == 1)
                            col = (m % 4) * 128
                            nc.tensor.matmul(
                                g[:, col:col + 128],
                                ch.whh[:, (2 * m + k) * 128:(2 * m + k + 1) * 128],
                                ch.src[k],
                                start=False, stop=last)
            ch.gA, ch.gB = gA, gB
            ch.gA_tile = gA

        # ---- phase 2 (final linear) groups per chain pair ----
        movs = {}
        for p, (Xf, Xb) in enumerate(((Xs[0], Xs[1]), (Xs[2], Xs[3]))):
            movs[p] = [X[:, k * XW:k * XW + CHUNK * 128].rearrange(
                           "p (t l) -> p t l", l=128)
                       for X in (Xf, Xb) for k in (0, 1)]
        gstate = {"gi": 0}

        def emit_group(p, p0, glen):
            n = glen * 128
            # Reuse the current step's (already activated) gate tile of a
            # rotating chain as the phase-2 PSUM accumulator: no pool
            # cycling, so the next step's matmuls never stall on the
            # bias-add. start=True re-zeroes the region (sigma already
            # consumed it).
            ps = chains[gstate["gi"] % 4].gA_tile[0:MEL, :]
            for k in range(4):
                nc.tensor.matmul(ps[:, 0:n], lin_w[:, k * MEL:(k + 1) * MEL],
                                 movs[p][k][:, p0:p0 + glen],
                                 start=(k == 0), stop=(k == 3))
            o_sb = ostage.tile([MEL, 512], F32, tag="os", name=f"os{p}_{p0}")
            nc.vector.tensor_scalar(o_sb[:, 0:n], ps[:, 0:n], lin_b[:], None,
                                    ADD)
            q = (nc.sync, nc.gpsimd)[gstate["gi"] % 2]
            q.dma_start(out_d[:, p, p0:p0 + glen], o_sb[:, 0:n])
            gstate["gi"] += 1

        # Late-ready position ranges (bwd finishes pos 0 last, fwd finishes
        # pos CHUNK-1 last) use small groups so little work serializes after
        # the final step.
        groups_at = {}
        glens = [(0, 2), (2, 2), (4, 4), (8, 4), (12, 4), (16, 2), (18, 2),
                 (20, 1)]
        for p in (0, 1):
            for p0, glen in glens:
                ready = W + max(p0 + glen - 1, CHUNK - 1 - p0)
                groups_at.setdefault(min(ready + (p % 2), K_STEPS - 1),
                                     []).append((p, p0, glen))

        def emit_sf(ch, s):
            sf = actp.tile([128, 512], BF16, tag="sf" + ch.name,
                           name=f"sf{ch.name}{s}")
            nc.scalar.activation(sf[:], ch.gA[:], SIG)
            ch.sf = sf

        def emit_tB(ch, s):
            t = actp.tile([128, 512], BF16, tag="tB" + ch.name,
                          name=f"tB{ch.name}{s}")
            nc.scalar.activation(t[:], ch.gB[:], TANH, scale=0.5)
            ch.tB = t

        def emit_fc(ch, s):
            fc = scr.tile([128, 256], BF16, tag="fc" + ch.name,
                          name=f"fc{ch.name}{s}")
            nc.vector.tensor_mul(fc[:], ch.sf[:, 256:512], ch.c_prev[:])
            ch.fc = fc

        def emit_ig(ch, s):
            ig = scr.tile([128, 256], BF16, tag="ig" + ch.name,
                          name=f"ig{ch.name}{s}")
            nc.vector.tensor_mul(ig[:], ch.sf[:, 0:256], ch.tB[:, 0:256])
            ch.ig = ig

        cpair = {}

        def emit_cnew(ch, s, pair, side):
            # both chains of a pair write one [128,512] c tile so tanh(c)
            # runs as ONE merged act per pair.
            if side == 0:
                cpair[pair] = state.tile([128, 512], BF16, tag=f"cP{pair}",
                                         name=f"cP{pair}_{s}")
            c_new = cpair[pair][:, side * 256:(side + 1) * 256]
            nc.vector.tensor_add(c_new, ch.fc[:], ch.ig[:])
            ch.c_new = c_new

        def emit_tc_pair(pair, s, ch_a, ch_b):
            tc_ = actp.tile([128, 512], BF16, tag=f"tcP{pair}",
                            name=f"tcP{pair}_{s}")
            nc.scalar.activation(tc_[:], cpair[pair][:], TANH)
            ch_a.tc = tc_[:, 0:256]
            ch_b.tc = tc_[:, 256:512]

        def emit_h(ch, s):
            real = s >= W
            t_rel = s - W
            if real:
                lp = t_rel if ch.name[0] == "F" else CHUNK - 1 - t_rel
            else:
                lp = CHUNK + (s & 1)
            dst = tuple(ch.X[:, k * XW + lp * 128:k * XW + (lp + 1) * 128]
                        for k in (0, 1))
            # h2 = (tanh(o/2) + 1) * tanh(c) = 2*sigma(o)*tanh(c); k-half
            # writes so next step's k0 recurrent mms start after half lands.
            for k in (0, 1):
                nc.vector.scalar_tensor_tensor(
                    dst[k], ch.tB[:, 256 + k * 128:256 + (k + 1) * 128], 1.0,
                    ch.tc[:, k * 128:(k + 1) * 128], ADD, MULT)
            ch.src = dst
            ch.c_prev = ch.c_new

        for s in range(K_STEPS):
            emit_dma(s + 1)
            for ch in chains:
                emit_mms(ch, s)
            emit_dma(s + 2)

            # Act queue: sfFA tBFA sfBA tBBA tcP0 sfFB tBFB sfBB tBBB tcP1;
            # DVE trails each chain's acts; h writes follow the pair's tc.
            c0, c1, c2, c3 = chains
            emit_sf(c0, s)
            emit_tB(c0, s)
            emit_sf(c1, s)
            emit_fc(c0, s)
            emit_ig(c0, s)
            emit_cnew(c0, s, 0, 0)
            emit_tB(c1, s)
            emit_fc(c1, s)
            emit_ig(c1, s)
            emit_cnew(c1, s, 0, 1)
            emit_tc_pair(0, s, c0, c1)
            emit_sf(c2, s)
            emit_tB(c2, s)
            emit_fc(c2, s)
            emit_ig(c2, s)
            emit_cnew(c2, s, 1, 0)
            emit_h(c0, s)
            emit_h(c1, s)
            emit_sf(c3, s)
            emit_tB(c3, s)
            emit_fc(c3, s)
            emit_ig(c3, s)
            emit_cnew(c3, s, 1, 1)
            emit_tc_pair(1, s, c2, c3)
            emit_h(c2, s)
            emit_h(c3, s)

            for (p, p0g, gl) in groups_at.get(s, []):
                emit_group(p, p0g, gl)

    nc.compile()
    return nc


def _np_lstm_fallback(exp, inputs):
    def sigmoid(z):
        return 1.0 / (1.0 + np.exp(-z))

    def lstm(xs, wih, whh, bih, bhh):
        Bb, L, E = xs.shape
        pre = np.einsum("ble,ge->blg", xs, wih) + bih + bhh
        h = np.zeros((Bb, HID), np.float32)
        c = np.zeros((Bb, HID), np.float32)
        hs = np.zeros((Bb, L, HID), np.float32)
        for t in range(L):
            gg = pre[:, t] + h @ whh.T
            i, f, g_, o = np.split(gg, 4, axis=-1)
            c = sigmoid(f) * c + sigmoid(i) * np.tanh(g_)
            h = sigmoid(o) * np.tanh(c)
            hs[:, t] = h
        return hs

    out_f = lstm(exp, inputs["wih_f"], inputs["whh_f"], inputs["bih_f"],
                 inputs["bhh_f"])
    out_b = lstm(exp[:, ::-1], inputs["wih_b"], inputs["whh_b"],
                 inputs["bih_b"], inputs["bhh_b"])[:, ::-1]
    out = np.concatenate([out_f, out_b], axis=-1)
    return out @ inputs["lin_w"].T + inputs["lin_b"]


def make_in_maps(expP, expR, inputs):
    import ml_dtypes
    bf16 = ml_dtypes.bfloat16
    rows, sc_ih, sc_hh = _mchunk_rows()

    def stat_tiles(w, scale):
        wp = (w.astype(np.float32)[rows] * scale[:, None])
        nk = w.shape[1] // 128
        out = np.zeros((128, 8 * nk * 128), np.float32)
        for m in range(8):
            for k in range(nk):
                out[:, (m * nk + k) * 128:(m * nk + k + 1) * 128] = \
                    wp[m * 128:(m + 1) * 128, k * 128:(k + 1) * 128].T
        return np.ascontiguousarray(out).astype(bf16)

    whhT_f = stat_tiles(inputs["whh_f"], sc_hh)
    whhT_b = stat_tiles(inputs["whh_b"], sc_hh)
    wihT_f = stat_tiles(inputs["wih_f"], sc_ih)
    wihT_b = stat_tiles(inputs["wih_b"], sc_ih)
    lw = inputs["lin_w"].astype(np.float32) * 0.5
    linT = np.concatenate([np.ascontiguousarray(lw[:, k * 128:(k + 1) * 128].T)
                           for k in range(4)], axis=1).astype(bf16)
    lin_b2 = np.ascontiguousarray(inputs["lin_b"].astype(np.float32)[:, None])

    in_maps = []
    for j in range(N_CORES):
        xein = np.zeros((K_STEPS, EMB, 512), np.float32)
        # lane blocks of 64: FA=(4j,4j+1), FB=(4j+2,4j+3),
        # BA=(31-4j,30-4j), BB=(29-4j,28-4j); xe cols FA 0:128, FB 128:256,
        # BA 256:384, BB 384:512
        cks = [4 * j, 4 * j + 1, 4 * j + 2, 4 * j + 3,
               31 - 4 * j, 30 - 4 * j, 29 - 4 * j, 28 - 4 * j]
        srcs = [expP] * 4 + [expR] * 4
        for s in range(K_STEPS):
            for ci, (ck, src) in enumerate(zip(cks, srcs)):
                p = ck * CHUNK - W + s
                if 0 <= p < L_PAD:
                    xein[s, :, ci * 64:(ci + 1) * 64] = src[:, p].T
        in_maps.append({
            "xein": xein.astype(bf16),
            "whhT_f": whhT_f, "whhT_b": whhT_b,
            "wihT_f": wihT_f, "wihT_b": wihT_b,
            "linT": linT, "lin_b": lin_b2,
        })
    return in_maps


def kernel(**inputs):
    global _COMPILED
    inputs = {k: np.asarray(v) for k, v in inputs.items()}
    x = inputs["x"].astype(np.int64)
    exp, L = _host_expand(x, inputs["embed"].astype(np.float32),
                          inputs["dp_w"].astype(np.float32),
                          inputs["dp_b"].astype(np.float32))

    bias_mag = max(float(np.abs(inputs[k]).max())
                   for k in ("bih_f", "bhh_f", "bih_b", "bhh_b"))
    if L > L_PAD or bias_mag != 0.0:
        f32in = {k: (v.astype(np.float32) if v.dtype.kind == "f" else v)
                 for k, v in inputs.items()}
        return _np_lstm_fallback(exp, f32in).astype(np.float32)

    expP = np.zeros((B, L_PAD, EMB), np.float32)
    expP[:, :L] = exp
    expR = expP[:, ::-1]

    in_maps = make_in_maps(expP, expR, inputs)

    if _COMPILED is None:
        _COMPILED = _build_kernel()
    nc = _COMPILED

    res = run_bass_kernel_spmd(nc, in_maps, core_ids=list(range(N_CORES)))

    out = np.empty((B, L_PAD, MEL), np.float32)
    for j in range(N_CORES):
        om = res.results[j]["out_mel"]          # [MEL, 2, CHUNK, 2, B]
        for p in (0, 1):
            for half in (0, 1):
                seg = om[:, p, :, half, :]      # [MEL, CHUNK, B]
                c = 4 * j + 2 * p + half
                out[:, c * CHUNK:(c + 1) * CHUNK] = seg.transpose(2, 1, 0)
    return np.ascontiguousarray(out[:, :L])


if __name__ == "__main__":
    inputs = dict(np.load("/root/problem/inputs.npz"))
    out = kernel(**inputs)
    ref = np.load("/root/problem/expected.npy")
    diff = np.abs(out - ref)
    print("out", out.shape, "absmax diff", diff.max(),
          "rel", diff.max() / np.abs(ref).max())


# revision 37
# speedup vs baseline: 1.1297x; 1.0092x over previous
"""MiniFastSpeech Trainium2 kernel (v6: 4-chain latency-hiding bf16 LSTM).

v3 (2 chains/core) measured loop-bound: the per-step recurrence
dependency chain (mms -> sigmoid -> DVE c-update -> tanh(c) -> h-write ->
mms) is ~4.5us while engine busy is only ~3.2us/step -- ~1.8us/step of
semaphore/pipeline dead time that scheduling cannot remove (every
DVE-produced value costs ~420ns to reach its consumer).

v6 goes busy-bound instead: 4 chains per core (2 fwd + 2 bwd, each 128
lanes = 2 seq-chunks x 64 batch; 32 chunks per direction, CHUNK=21,
W=12 warmup). The period must cover 4 chains' engine work (~6.4us on
Act) which exceeds the ~4.5us chain loop, so the recurrence latency
hides completely. Act work per chain-step: sigmoid [512] over bank A =
[i,f], tanh(x/2) [512] over bank B = [g,o] (g rows pre-doubled -> exact
tanh(g)), tanh [256] of c. DVE (bf16 2x): fc, ig, c_new tensor ops +
2 scalar_tensor_tensor h-writes computing h2 = (tanh(o/2)+1)*tanh(c) =
2h (whh/lin pre-halved on host absorb the 2x). PSUM: 8 banks = 4 chains
x 2 banks, bufs=1; xe matmuls run in-step (no prefetch; PE has slack).
Phase-2 final linear per chain-pair interleaved into the loop; bias add
on Pool.
"""

import sys
import numpy as np
from contextlib import ExitStack

sys.path.insert(0, "/opt/trn_rl_repo")

import concourse.bass as bass
import concourse.tile as tile
from concourse import bacc, mybir
from concourse.bass_utils import run_bass_kernel_spmd

# ---- problem constants (hardcoded per contract) ----
VOCAB, EMB, HID, MEL = 256, 128, 256, 80
B, T = 64, 512
N_CORES = 8
NCHUNK = 32          # chunks per direction
W = 11               # warmup steps per chain
CHUNK = 21           # positions per chunk; L_PAD = 672 >= L
L_PAD = NCHUNK * CHUNK
K_STEPS = W + CHUNK
F32 = mybir.dt.float32
BF16 = mybir.dt.bfloat16
SIG = mybir.ActivationFunctionType.Sigmoid
TANH = mybir.ActivationFunctionType.Tanh
MULT = mybir.AluOpType.mult
ADD = mybir.AluOpType.add

_COMPILED = None


def _host_expand(x, embed, dp_w, dp_b):
    xe = embed[x]                                   # (B,T,E)
    d = np.maximum(xe @ dp_w[0] + dp_b[0], 0)
    dur = np.floor(d).astype(np.int64) + 1
    cum = np.cumsum(dur, axis=1)
    L = int(cum[:, -1].max())
    pos = np.arange(L)
    idx = np.empty((B, L), np.int64)
    for b in range(B):
        idx[b] = np.searchsorted(cum[b], pos, side="right")
    mask = (pos[None, :] < cum[:, -1:]).astype(np.float32)
    exp = np.take_along_axis(xe, np.clip(idx, 0, T - 1)[..., None], axis=1)
    return np.ascontiguousarray(exp * mask[..., None], dtype=np.float32), L


# m-chunk order [i0 i1 f0 f1 | g0 g1 o0 o1]; rows in PyTorch [i,f,g,o] layout.
# Bank A = [i,f] -> one sigmoid act [512]. Bank B = [g,o] -> one tanh(x*0.5)
# act: g rows pre-doubled -> exact tanh(g) in cols 0:256; o gives tanh(o/2).
# whh: additionally all rows halved because the moving h operand is 2h.
def _mchunk_rows():
    rows, sc_ih, sc_hh = [], [], []
    for base, sc in ((0, 1.0), (HID, 1.0), (2 * HID, 2.0), (3 * HID, 1.0)):
        for half in (0, 1):
            rows.append(np.arange(base + half * 128, base + half * 128 + 128))
            sc_ih.append(np.full(128, sc, np.float32))
            sc_hh.append(np.full(128, sc * 0.5, np.float32))
    return (np.concatenate(rows), np.concatenate(sc_ih), np.concatenate(sc_hh))


class _Chain:
    def __init__(self, name, whh, wih, xe_cols, X, poolA, poolB):
        self.name = name
        self.whh = whh          # sbuf [128, 16*128] bf16, tile (m,k) at (2m+k)*128
        self.wih = wih          # sbuf [128, 8*128] bf16, tile m at m*128
        self.xe_cols = xe_cols  # slice in the xein tile
        self.X = X              # sbuf [128, 2*XW] bf16; k-half at k*XW
        self.poolA = poolA
        self.poolB = poolB
        self.gA = None
        self.gB = None
        self.src = None         # (h0, h1) col blocks [128,128] (2h of prev step)
        self.c_prev = None
        self.sf = None
        self.tB = None
        self.fc = None
        self.ig = None
        self.c_new = None
        self.tc = None


def _build_kernel():
    nc = bacc.Bacc("TRN2", target_bir_lowering=False, debug=False,
                   num_devices=N_CORES)

    xein = nc.dram_tensor("xein", [K_STEPS, EMB, 512], BF16,
                          kind="ExternalInput").ap()
    whh_f_d = nc.dram_tensor("whhT_f", [128, 16 * 128], BF16, kind="ExternalInput").ap()
    whh_b_d = nc.dram_tensor("whhT_b", [128, 16 * 128], BF16, kind="ExternalInput").ap()
    wih_f_d = nc.dram_tensor("wihT_f", [128, 8 * 128], BF16, kind="ExternalInput").ap()
    wih_b_d = nc.dram_tensor("wihT_b", [128, 8 * 128], BF16, kind="ExternalInput").ap()
    lin_w_d = nc.dram_tensor("linT", [128, 4 * MEL], BF16, kind="ExternalInput").ap()
    lin_b_d = nc.dram_tensor("lin_b", [MEL, 1], F32, kind="ExternalInput").ap()
    out_d = nc.dram_tensor("out_mel", [MEL, 2, CHUNK, 2, B], F32,
                           kind="ExternalOutput").ap()

    with tile.TileContext(nc) as tc, ExitStack() as ctx:
        wpool = ctx.enter_context(tc.tile_pool(name="weights", bufs=1))
        xpool = ctx.enter_context(tc.tile_pool(name="xstream", bufs=4))
        state = ctx.enter_context(tc.tile_pool(name="state", bufs=3))
        actp = ctx.enter_context(tc.tile_pool(name="acts", bufs=3))
        xbig = ctx.enter_context(tc.tile_pool(name="xbig", bufs=1))
        scr = ctx.enter_context(tc.tile_pool(name="scratch", bufs=3))
        psA = [ctx.enter_context(tc.tile_pool(name=f"gA{i}", bufs=1,
                                              space="PSUM")) for i in range(4)]
        psB = [ctx.enter_context(tc.tile_pool(name=f"gB{i}", bufs=1,
                                              space="PSUM")) for i in range(4)]
        ostage = ctx.enter_context(tc.tile_pool(name="ostage", bufs=4))

        # ---- memsets first (Pool queue) so the PE pre-warm starts at t~0
        hinit = wpool.tile([128, 256], BF16, tag="hinit")
        nc.gpsimd.memset(hinit[:], 0.0)
        zstat_bf = wpool.tile([128, 64], BF16, tag="zstatbf")
        nc.gpsimd.memset(zstat_bf[:], 0.0)

        # PE p-state pre-warm: burn the ramp on dummy matmuls while the
        # weight DMAs are in flight, so step 0 runs at full clock.
        warm = psB[3].tile([128, 512], F32, tag="g", name="pewarm")
        NWARM = 10
        for i in range(NWARM):
            nc.tensor.matmul(warm[0:64, 0:256], zstat_bf[:], hinit[:],
                             start=(i == 0), stop=(i == NWARM - 1))

        # ---- xe stream DMAs ----
        xe_tiles = {}

        def emit_dma(s):
            if s not in xe_tiles and s < K_STEPS:
                xe = xpool.tile([EMB, 512], BF16, tag="xe", name=f"xe{s}")
                nc.sync.dma_start(xe[:], xein[s])
                xe_tiles[s] = xe

        # ---- weights -> SBUF; wih_f first (it gates the first sigmoid)
        wih_f = wpool.tile([128, 8 * 128], BF16, tag="wihf")
        nc.sync.dma_start(wih_f[:], wih_f_d[:])
        emit_dma(0)
        emit_dma(1)
        wih_b = wpool.tile([128, 8 * 128], BF16, tag="wihb")
        nc.gpsimd.dma_start(wih_b[:], wih_b_d[:])
        whh_f = wpool.tile([128, 16 * 128], BF16, tag="whhf")
        nc.sync.dma_start(whh_f[:], whh_f_d[:])
        whh_b = wpool.tile([128, 16 * 128], BF16, tag="whhb")
        nc.scalar.dma_start(whh_b[:], whh_b_d[:])
        lin_w = wpool.tile([128, 4 * MEL], BF16, tag="linw")
        nc.scalar.dma_start(lin_w[:], lin_w_d[:])
        lin_b = wpool.tile([MEL, 1], F32, tag="linb")
        nc.gpsimd.dma_start(lin_b[:], lin_b_d[:])

        XW = (CHUNK + 2) * 128
        Xs = [xbig.tile([128, 2 * XW], BF16, tag=f"X{i}", name=f"X{i}")
              for i in range(4)]

        # chains: FA, BA, FB, BB (emit order); xe col blocks FA 0:128,
        # FB 128:256, BA 256:384, BB 384:512
        chains = [
            _Chain("FA", whh_f, wih_f, slice(0, 128), Xs[0], psA[0], psB[0]),
            _Chain("BA", whh_b, wih_b, slice(256, 384), Xs[1], psA[1], psB[1]),
            _Chain("FB", whh_f, wih_f, slice(128, 256), Xs[2], psA[2], psB[2]),
            _Chain("BB", whh_b, wih_b, slice(384, 512), Xs[3], psA[3], psB[3]),
        ]
        for ch in chains:
            ch.src = (hinit[:, 0:128], hinit[:, 128:256])
            c0 = state.tile([128, 256], BF16, tag="c" + ch.name,
                            name=f"c0{ch.name}")
            nc.gpsimd.memset(c0[:], 0.0)
            ch.c_prev = c0

        def emit_mms(ch, s):
            # all of bank A (xe + rec, k-major), then bank B. bufs=1: the
            # tile from step s-1 is recycled; Tile waits on its readers.
            xe = xe_tiles[s]
            gA = ch.poolA.tile([128, 512], F32, tag="g", name=f"gA{ch.name}{s}")
            gB = ch.poolB.tile([128, 512], F32, tag="g", name=f"gB{ch.name}{s}")
            for g, mbase in ((gA, 0), (gB, 4)):
                for m in range(mbase, mbase + 4):
                    col = (m % 4) * 128
                    nc.tensor.matmul(g[:, col:col + 128],
                                     ch.wih[:, m * 128:(m + 1) * 128],
                                     xe[:, ch.xe_cols],
                                     start=(m == mbase),
                                     stop=(s == 0 and m == mbase + 3))
                if s > 0:
                    for k in (0, 1):
                        for m in range(mbase, mbase + 4):
                            last = (m == mbase + 3) and (k == 1)
                            col = (m % 4) * 128
                            nc.tensor.matmul(
                                g[:, col:col + 128],
                                ch.whh[:, (2 * m + k) * 128:(2 * m + k + 1) * 128],
                                ch.src[k],
                                start=False, stop=last)
            ch.gA, ch.gB = gA, gB
            ch.gA_tile = gA

        # ---- phase 2 (final linear) groups per chain pair ----
        movs = {}
        for p, (Xf, Xb) in enumerate(((Xs[0], Xs[1]), (Xs[2], Xs[3]))):
            movs[p] = [X[:, k * XW:k * XW + CHUNK * 128].rearrange(
                           "p (t l) -> p t l", l=128)
                       for X in (Xf, Xb) for k in (0, 1)]
        gstate = {"gi": 0}

        def emit_group(p, p0, glen):
            n = glen * 128
            # Reuse the current step's (already activated) gate tile of a
            # rotating chain as the phase-2 PSUM accumulator: no pool
            # cycling, so the next step's matmuls never stall on the
            # bias-add. start=True re-zeroes the region (sigma already
            # consumed it).
            ps = chains[gstate["gi"] % 4].gA_tile[0:MEL, :]
            for k in range(4):
                nc.tensor.matmul(ps[:, 0:n], lin_w[:, k * MEL:(k + 1) * MEL],
                                 movs[p][k][:, p0:p0 + glen],
                                 start=(k == 0), stop=(k == 3))
            o_sb = ostage.tile([MEL, 512], F32, tag="os", name=f"os{p}_{p0}")
            nc.vector.tensor_scalar(o_sb[:, 0:n], ps[:, 0:n], lin_b[:], None,
                                    ADD)
            q = (nc.sync, nc.gpsimd)[gstate["gi"] % 2]
            q.dma_start(out_d[:, p, p0:p0 + glen], o_sb[:, 0:n])
            gstate["gi"] += 1

        # Late-ready position ranges (bwd finishes pos 0 last, fwd finishes
        # pos CHUNK-1 last) use small groups so little work serializes after
        # the final step.
        groups_at = {}
        glens = [(0, 2), (2, 2), (4, 4), (8, 4), (12, 4), (16, 2), (18, 2),
                 (20, 1)]
        for p in (0, 1):
            for p0, glen in glens:
                ready = W + max(p0 + glen - 1, CHUNK - 1 - p0)
                groups_at.setdefault(min(ready + (p % 2), K_STEPS - 1),
                                     []).append((p, p0, glen))

        def emit_sf(ch, s):
            sf = actp.tile([128, 512], BF16, tag="sf" + ch.name,
                           name=f"sf{ch.name}{s}")
            nc.scalar.activation(sf[:], ch.gA[:], SIG)
            ch.sf = sf

        def emit_tB(ch, s):
            t = actp.tile([128, 512], BF16, tag="tB" + ch.name,
                          name=f"tB{ch.name}{s}")
            nc.scalar.activation(t[:], ch.gB[:], TANH, scale=0.5)
            ch.tB = t

        def emit_fc(ch, s):
            fc = scr.tile([128, 256], BF16, tag="fc" + ch.name,
                          name=f"fc{ch.name}{s}")
            nc.vector.tensor_mul(fc[:], ch.sf[:, 256:512], ch.c_prev[:])
            ch.fc = fc

        def emit_ig(ch, s):
            ig = scr.tile([128, 256], BF16, tag="ig" + ch.name,
                          name=f"ig{ch.name}{s}")
            nc.vector.tensor_mul(ig[:], ch.sf[:, 0:256], ch.tB[:, 0:256])
            ch.ig = ig

        cpair = {}

        def emit_cnew(ch, s, pair, side):
            # both chains of a pair write one [128,512] c tile so tanh(c)
            # runs as ONE merged act per pair.
            if side == 0:
                cpair[pair] = state.tile([128, 512], BF16, tag=f"cP{pair}",
                                         name=f"cP{pair}_{s}")
            c_new = cpair[pair][:, side * 256:(side + 1) * 256]
            nc.vector.tensor_add(c_new, ch.fc[:], ch.ig[:])
            ch.c_new = c_new

        def emit_tc_pair(pair, s, ch_a, ch_b):
            tc_ = actp.tile([128, 512], BF16, tag=f"tcP{pair}",
                            name=f"tcP{pair}_{s}")
            nc.scalar.activation(tc_[:], cpair[pair][:], TANH)
            ch_a.tc = tc_[:, 0:256]
            ch_b.tc = tc_[:, 256:512]

        def emit_h(ch, s):
            real = s >= W
            t_rel = s - W
            if real:
                lp = t_rel if ch.name[0] == "F" else CHUNK - 1 - t_rel
            else:
                lp = CHUNK + (s & 1)
            dst = tuple(ch.X[:, k * XW + lp * 128:k * XW + (lp + 1) * 128]
                        for k in (0, 1))
            # h2 = (tanh(o/2) + 1) * tanh(c) = 2*sigma(o)*tanh(c); k-half
            # writes so next step's k0 recurrent mms start after half lands.
            for k in (0, 1):
                nc.vector.scalar_tensor_tensor(
                    dst[k], ch.tB[:, 256 + k * 128:256 + (k + 1) * 128], 1.0,
                    ch.tc[:, k * 128:(k + 1) * 128], ADD, MULT)
            ch.src = dst
            ch.c_prev = ch.c_new

        for s in range(K_STEPS):
            emit_dma(s + 1)
            for ch in chains:
                emit_mms(ch, s)
            emit_dma(s + 2)

            # Act queue: sfFA tBFA sfBA tBBA tcP0 sfFB tBFB sfBB tBBB tcP1;
            # DVE trails each chain's acts; h writes follow the pair's tc.
            c0, c1, c2, c3 = chains
            emit_sf(c0, s)
            emit_tB(c0, s)
            emit_sf(c1, s)
            emit_fc(c0, s)
            emit_ig(c0, s)
            emit_cnew(c0, s, 0, 0)
            emit_tB(c1, s)
            emit_fc(c1, s)
            emit_ig(c1, s)
            emit_cnew(c1, s, 0, 1)
            emit_tc_pair(0, s, c0, c1)
            emit_sf(c2, s)
            emit_tB(c2, s)
            emit_fc(c2, s)
            emit_ig(c2, s)
            emit_cnew(c2, s, 1, 0)
            emit_h(c0, s)
            emit_h(c1, s)
            emit_sf(c3, s)
            emit_tB(c3, s)
            emit_fc(c3, s)
            emit_ig(c3, s)
            emit_cnew(c3, s, 1, 1)
            emit_tc_pair(1, s, c2, c3)
            emit_h(c2, s)
            emit_h(c3, s)

            for (p, p0g, gl) in groups_at.get(s, []):
                emit_group(p, p0g, gl)

    nc.compile()
    return nc


def _np_lstm_fallback(exp, inputs):
    def sigmoid(z):
        return 1.0 / (1.0 + np.exp(-z))

    def lstm(xs, wih, whh, bih, bhh):
        Bb, L, E = xs.shape
        pre = np.einsum("ble,ge->blg", xs, wih) + bih + bhh
        h = np.zeros((Bb, HID), np.float32)
        c = np.zeros((Bb, HID), np.float32)
        hs = np.zeros((Bb, L, HID), np.float32)
        for t in range(L):
            gg = pre[:, t] + h @ whh.T
            i, f, g_, o = np.split(gg, 4, axis=-1)
            c = sigmoid(f) * c + sigmoid(i) * np.tanh(g_)
            h = sigmoid(o) * np.tanh(c)
            hs[:, t] = h
        return hs

    out_f = lstm(exp, inputs["wih_f"], inputs["whh_f"], inputs["bih_f"],
                 inputs["bhh_f"])
    out_b = lstm(exp[:, ::-1], inputs["wih_b"], inputs["whh_b"],
                 inputs["bih_b"], inputs["bhh_b"])[:, ::-1]
    out = np.concatenate([out_f, out_b], axis=-1)
    return out @ inputs["lin_w"].T + inputs["lin_b"]


def make_in_maps(expP, expR, inputs):
    import ml_dtypes
    bf16 = ml_dtypes.bfloat16
    rows, sc_ih, sc_hh = _mchunk_rows()

    def stat_tiles(w, scale):
        wp = (w.astype(np.float32)[rows] * scale[:, None])
        nk = w.shape[1] // 128
        out = np.zeros((128, 8 * nk * 128), np.float32)
        for m in range(8):
            for k in range(nk):
                out[:, (m * nk + k) * 128:(m * nk + k + 1) * 128] = \
                    wp[m * 128:(m + 1) * 128, k * 128:(k + 1) * 128].T
        return np.ascontiguousarray(out).astype(bf16)

    whhT_f = stat_tiles(inputs["whh_f"], sc_hh)
    whhT_b = stat_tiles(inputs["whh_b"], sc_hh)
    wihT_f = stat_tiles(inputs["wih_f"], sc_ih)
    wihT_b = stat_tiles(inputs["wih_b"], sc_ih)
    lw = inputs["lin_w"].astype(np.float32) * 0.5
    linT = np.concatenate([np.ascontiguousarray(lw[:, k * 128:(k + 1) * 128].T)
                           for k in range(4)], axis=1).astype(bf16)
    lin_b2 = np.ascontiguousarray(inputs["lin_b"].astype(np.float32)[:, None])

    in_maps = []
    for j in range(N_CORES):
        xein = np.zeros((K_STEPS, EMB, 512), np.float32)
        # lane blocks of 64: FA=(4j,4j+1), FB=(4j+2,4j+3),
        # BA=(31-4j,30-4j), BB=(29-4j,28-4j); xe cols FA 0:128, FB 128:256,
        # BA 256:384, BB 384:512
        cks = [4 * j, 4 * j + 1, 4 * j + 2, 4 * j + 3,
               31 - 4 * j, 30 - 4 * j, 29 - 4 * j, 28 - 4 * j]
        srcs = [expP] * 4 + [expR] * 4
        for s in range(K_STEPS):
            for ci, (ck, src) in enumerate(zip(cks, srcs)):
                p = ck * CHUNK - W + s
                if 0 <= p < L_PAD:
                    xein[s, :, ci * 64:(ci + 1) * 64] = src[:, p].T
        in_maps.append({
            "xein": xein.astype(bf16),
            "whhT_f": whhT_f, "whhT_b": whhT_b,
            "wihT_f": wihT_f, "wihT_b": wihT_b,
            "linT": linT, "lin_b": lin_b2,
        })
    return in_maps


def kernel(**inputs):
    global _COMPILED
    inputs = {k: np.asarray(v) for k, v in inputs.items()}
    x = inputs["x"].astype(np.int64)
    exp, L = _host_expand(x, inputs["embed"].astype(np.float32),
                          inputs["dp_w"].astype(np.float32),
                          inputs["dp_b"].astype(np.float32))

    bias_mag = max(float(np.abs(inputs[k]).max())
                   for k in ("bih_f", "bhh_f", "bih_b", "bhh_b"))
    if L > L_PAD or bias_mag != 0.0:
        f32in = {k: (v.astype(np.float32) if v.dtype.kind == "f" else v)
                 for k, v in inputs.items()}
        return _np_lstm_fallback(exp, f32in).astype(np.float32)

    expP = np.zeros((B, L_PAD, EMB), np.float32)
    expP[:, :L] = exp
    expR = expP[:, ::-1]

    in_maps = make_in_maps(expP, expR, inputs)

    if _COMPILED is None:
        _COMPILED = _build_kernel()
    nc = _COMPILED

    res = run_bass_kernel_spmd(nc, in_maps, core_ids=list(range(N_CORES)))

    out = np.empty((B, L_PAD, MEL), np.float32)
    for j in range(N_CORES):
        om = res.results[j]["out_mel"]          # [MEL, 2, CHUNK, 2, B]
        for p in (0, 1):
            for half in (0, 1):
                seg = om[:, p, :, half, :]      # [MEL, CHUNK, B]
                c = 4 * j + 2 * p + half
                out[:, c * CHUNK:(c + 1) * CHUNK] = seg.transpose(2, 1, 0)
    return np.ascontiguousarray(out[:, :L])


if __name__ == "__main__":
    inputs = dict(np.load("/root/problem/inputs.npz"))
    out = kernel(**inputs)
    ref = np.load("/root/problem/expected.npy")
    diff = np.abs(out - ref)
    print("out", out.shape, "absmax diff", diff.max(),
          "rel", diff.max() / np.abs(ref).max())
